# revision 26
# baseline (speedup 1.0000x reference)
"""Trainium2 Bass kernel for a 2-attention-block + FFN decoder stack.

Shapes: x (4, 2048, 768), 12 heads x 64, d_ff 3072.
Sharding over 8 cores: core c handles batch b=c//2 and heads 6*(c%2)..+6 for
both attention blocks; the final FFN+LN runs on token half c%2 of batch b.
Per-pair bf16 AllGathers (replica groups [[0,1],[2,3],...]) exchange the
per-head attention outputs so each core can LayerNorm over the full model
dim.

All compute is done in "transposed" layout (D on partitions, tokens on the
free axis).  The source model's softmax runs over the *query* axis (dim=2
quirk), which in transposed layout (k on partitions, q on free axis) is a
per-partition-row softmax: exp on ScalarE with accum_out produces the row
sums for free; the 1/rowsum is folded into the (tiny) KV matrix instead of
the (huge) score matrix.  No max-subtraction is needed: |w| stays O(10) so
exp cannot overflow, and softmax is shift-invariant.

Block-1 exploits the causal mask: fully-masked 512-wide q-chunks of each
128-row k-tile are skipped entirely (no QK, no exp, no AV -- exp(-1e9)=0
contributes nothing to row sums or AV), and the additive mask matmul runs
only on the diagonal chunk.  kv in token layout comes from PE transposes of
kv^T instead of a second x@Wv matmul.  Projections accumulate with dt as
the outer loop so the preceding LayerNorm's per-tile outputs pipeline
straight into the next block's matmuls.
"""

import os
import sys

for _p in ("/opt/trn_rl_repo", "/root/.axon_site/_ro/trn_rl_repo"):
    if os.path.isdir(_p) and _p not in sys.path:
        sys.path.insert(0, _p)

import numpy as np
from contextlib import ExitStack

from concourse import bass, bacc, mybir, tile
from concourse import bass_utils

F32 = mybir.dt.float32
BF16 = mybir.dt.bfloat16
I32 = mybir.dt.int32
NP_BF16 = mybir.dt.np(BF16)

B, S, D, H, DH, DFF = 4, 2048, 768, 12, 64, 3072
NCORES = 8
HLOC = 6           # heads per core
NPAIR = 3          # head pairs per core
SQRT_DK = float(np.sqrt(DH))
EPS = 1e-5
SH = S // 2        # token half for FFN
DT = D // 128      # 6 d-tiles
KT = S // 128      # 16 k-tiles
QC = S // 512      # 4 q-chunks
FT = DFF // 128    # 24 ff-tiles

# pairwise replica groups: the two cores sharing a batch exchange heads
RG = [[0, 1], [2, 3], [4, 5], [6, 7]]
# LN input processing order: pairs 0,0,1,1,2,2 so the earliest-gathered
# pair's tiles are consumed first
DT_ORDER = [0, 3, 1, 4, 2, 5]

Exp = mybir.ActivationFunctionType.Exp
Sqrt = mybir.ActivationFunctionType.Sqrt
Add = mybir.AluOpType.add
Mult = mybir.AluOpType.mult
Max = mybir.AluOpType.max


def _scalar_from_input(nc, dram, max_val):
    tmp = nc.alloc_registers(f"sv_{dram.name}", mybir.ALL_ENGINES)
    nc.regs_load(tmp, dram[0:1, 0:1])
    return nc.snap(tmp, donate=True, min_val=0, max_val=max_val)


def _layernorm(tc, ctx, r_tiles, gb_sb, ones_b, width, out_f, out_b):
    """LayerNorm over the partition (D) axis of 6 x (128, width) tiles.

    r_tiles may be f32 or bf16; out_f (f32) and out_b (bf16) are optional
    lists of destination tiles.  gb_sb is a (2, 768) bf16 SBUF tile (row 0
    gamma, row 1 beta), applied via tiny outer-product matmuls building
    per-element affine maps.
    """
    nc = tc.nc
    ch_n = width // 512
    sb = ctx.enter_context(tc.tile_pool(name="ln_sb", bufs=1))
    # all row-vector scratch lives at base partition 0 (engine requirement)
    mu = sb.tile([1, width], F32, tag="ln_mu", name="ln_mu")
    msq = sb.tile([1, width], F32, tag="ln_msq", name="ln_msq")
    am = sb.tile([1, width], F32, tag="ln_am", name="ln_am")
    bm = sb.tile([2, width], F32, tag="ln_bm", name="ln_bm")
    nc.vector.memset(bm[0:2, :], 1.0)  # row1 stays ones; row0 overwritten

    with ExitStack() as sctx:
        sq_pool = sctx.enter_context(tc.tile_pool(name="ln_sq", bufs=2))
        ps_pool = sctx.enter_context(
            tc.tile_pool(name="ln_stats_ps", bufs=1, space="PSUM"))
        sum_ps = [ps_pool.tile([1, 512], F32, tag=f"sum{ch}", name=f"sum{ch}")
                  for ch in range(ch_n)]
        ssq_ps = [ps_pool.tile([1, 512], F32, tag=f"ssq{ch}", name=f"ssq{ch}")
                  for ch in range(ch_n)]
        for i, dt in enumerate(DT_ORDER):
            r = r_tiles[dt]
            if r.dtype == BF16:
                rb = r
            else:
                rb = sq_pool.tile([128, width], BF16, tag="rb", name="rb")
                nc.vector.tensor_copy(rb[:], r[:])
            sq = sq_pool.tile([128, width], BF16, tag="sq", name="sq")
            nc.vector.tensor_mul(sq[:], rb[:], rb[:])
            for ch in range(ch_n):
                cs = slice(512 * ch, 512 * ch + 512)
                nc.tensor.matmul(sum_ps[ch][:], ones_b[:, 0:1],
                                 rb[:, cs],
                                 start=(i == 0), stop=(i == DT - 1))
                nc.tensor.matmul(ssq_ps[ch][:], ones_b[:, 0:1],
                                 sq[:, cs],
                                 start=(i == 0), stop=(i == DT - 1))
        for ch in range(ch_n):
            cs = slice(512 * ch, 512 * ch + 512)
            nc.vector.tensor_scalar_mul(mu[0:1, cs], sum_ps[ch][:], 1.0 / D)
            nc.vector.tensor_scalar_mul(msq[0:1, cs], ssq_ps[ch][:], 1.0 / D)

    # var = msq - mu^2 ; sd = sqrt(var + eps) ; rstd = 1/sd ; -mu*rstd
    # chunked so early chunks' broadcast matmuls start before late chunks'
    # stats finish (cuts the serial row-chain latency out of the LN span)
    tmp = sb.tile([1, width], F32, tag="ln_tmp", name="ln_tmp")
    amb = sb.tile([1, width], BF16, tag="ln_amb", name="ln_amb")
    bmb = sb.tile([2, width], BF16, tag="ln_bmb", name="ln_bmb")
    for ch in range(ch_n):
        cs = slice(512 * ch, 512 * ch + 512)
        nc.vector.tensor_mul(tmp[0:1, cs], mu[0:1, cs], mu[0:1, cs])
        nc.vector.tensor_sub(msq[0:1, cs], msq[0:1, cs], tmp[0:1, cs])
        nc.vector.tensor_scalar_add(msq[0:1, cs], msq[0:1, cs], EPS)
        nc.scalar.activation(msq[0:1, cs], msq[0:1, cs], Sqrt)
        nc.vector.reciprocal(am[0:1, cs], msq[0:1, cs])
        nc.vector.scalar_tensor_tensor(bm[0:1, cs], mu[0:1, cs], -1.0,
                                       am[0:1, cs], op0=Mult, op1=Mult)
        nc.vector.tensor_copy(amb[0:1, cs], am[0:1, cs])
        nc.vector.tensor_copy(bmb[0:2, cs], bm[0:2, cs])

    # apply chunk-outer (512 cols of all 6 tiles at a time) so consumers of
    # the first output columns start long before the full apply finishes
    with (
        tc.tile_pool(name="ln_ab_ps", bufs=2, space="PSUM") as ab_pool,
        tc.tile_pool(name="ln_ap", bufs=2) as ap_pool,
    ):
        for ch in range(ch_n):
            cs = slice(512 * ch, 512 * ch + 512)
            for dt in DT_ORDER:
                amat = ab_pool.tile([128, 512], F32, tag="ln_amat",
                                    name="ln_amat")
                bmat = ab_pool.tile([128, 512], F32, tag="ln_bmat",
                                    name="ln_bmat")
                nc.tensor.matmul(amat[:],
                                 gb_sb[0:1, 128 * dt:128 * dt + 128],
                                 amb[0:1, cs], start=True, stop=True)
                nc.tensor.matmul(bmat[:],
                                 gb_sb[0:2, 128 * dt:128 * dt + 128],
                                 bmb[0:2, cs], start=True, stop=True)
                if out_f is not None:
                    dst = out_f[dt]
                    nc.vector.tensor_mul(dst[:, cs], r_tiles[dt][:, cs],
                                         amat[:])
                    nc.vector.tensor_add(dst[:, cs], dst[:, cs], bmat[:])
                    if out_b is not None:
                        nc.vector.tensor_copy(out_b[dt][:, cs],
                                              out_f[dt][:, cs])
                else:
                    # f32 intermediate: only one bf16 rounding on the output
                    tmpa = ap_pool.tile([128, 512], F32, tag="ln_apf",
                                        name="ln_apf")
                    nc.vector.tensor_mul(tmpa[:], r_tiles[dt][:, cs],
                                         amat[:])
                    nc.vector.tensor_add(out_b[dt][:, cs], tmpa[:], bmat[:])


def _attention(tc, ctx, x_tiles, wq_sb, wv_sb, mask_tiles, ag_in, ident_sb,
               causal, on_pair=None, dt_order=None):
    """One attention block in transposed layout (all-bf16 matmul operands).

    x_tiles: 6 x (128, S) bf16 SBUF tiles (caller-owned).
    causal=True skips fully-masked q-regions at 128-column granularity and
    applies mask_tiles (16 x (128, 128) bf16 additive diagonal-block mask)
    via identity-matmul accumulation into the score PSUM.
    dt_order: projection contraction order (to match the order the caller's
    x tiles become ready).
    Writes o^T for this core's 6 heads (384, S) bf16 into ag_in DRAM.
    """
    nc = tc.nc
    if dt_order is None:
        dt_order = list(range(DT))

    qkv_pool = ctx.enter_context(tc.tile_pool(name="attn_qkv", bufs=1))
    kv_pool = ctx.enter_context(tc.tile_pool(name="attn_kv", bufs=1))
    qt_sb = [qkv_pool.tile([128, S], BF16, tag=f"qt{p}", name=f"qt{p}")
             for p in range(NPAIR)]
    kvt_sb = [qkv_pool.tile([128, S], BF16, tag=f"kvt{p}", name=f"kvt{p}")
              for p in range(NPAIR)]
    kv_sb = [kv_pool.tile([128, NPAIR * 128], BF16, tag=f"kv{kt}",
                          name=f"kv{kt}") for kt in range(KT)]

    # projections, dt-outer so x tiles are consumed as they become ready
    with (
        tc.tile_pool(name="attn_proj_ps", bufs=1, space="PSUM") as pps,
        tc.tile_pool(name="attn_tr_ps", bufs=2, space="PSUM") as tps,
    ):
        for qc in range(QC):
            qs = slice(512 * qc, 512 * qc + 512)
            tiles = [pps.tile([128, 512], F32, tag=f"proj{j}",
                              name=f"proj{j}") for j in range(2 * NPAIR)]
            for i, dt in enumerate(dt_order):
                j = 0
                for p in range(NPAIR):
                    for wsb in (wq_sb, wv_sb):
                        nc.tensor.matmul(
                            tiles[j][:], wsb[dt][:, 128 * p:128 * p + 128],
                            x_tiles[dt][:, qs],
                            start=(i == 0), stop=(i == DT - 1))
                        j += 1
            j = 0
            for p in range(NPAIR):
                for dst in (qt_sb, kvt_sb):
                    nc.vector.tensor_copy(dst[p][:, qs], tiles[j][:])
                    j += 1
            # kv token-layout tiles via PE transpose of kv^T
            for kt in range(4 * qc, 4 * qc + 4):
                tp = tps.tile([128, NPAIR * 128], BF16, tag="tr", name="tr")
                for p in range(NPAIR):
                    nc.tensor.matmul(
                        tp[:, 128 * p:128 * p + 128],
                        kvt_sb[p][:, 128 * kt:128 * kt + 128],
                        ident_sb[:], is_transpose=True,
                        start=True, stop=True)
                nc.vector.tensor_copy(kv_sb[kt][:], tp[:])

    # attention proper, one head-pair at a time.
    # PSUM: ot (128,2048)f32 = 4 banks; wt (128,1024)f32 x 2 bufs = 4 banks.
    with (
        tc.tile_pool(name="attn_wt_ps", bufs=2, space="PSUM") as wt_pool,
        tc.tile_pool(name="attn_ot_ps", bufs=1, space="PSUM") as ot_pool,
        tc.tile_pool(name="attn_sc", bufs=3) as sc_pool,
        tc.tile_pool(name="attn_rs", bufs=8) as rs_pool,
        tc.tile_pool(name="attn_o", bufs=2) as o_pool,
    ):
        for p in range(NPAIR):
            ot = ot_pool.tile([128, S], F32, tag="ot", name="ot")
            for kt in range(KT):
                ksl = slice(128 * kt, 128 * kt + 128)
                dq = kt // 4 if causal else 0
                r128 = kt % 4 if causal else 0
                win = 128 * r128 + 128
                heads = {}
                for hi, (plo, phi) in enumerate(((0, 64), (64, 128))):
                    score = sc_pool.tile([128, S], BF16, tag=f"sc{hi}",
                                         name=f"sc{hi}")
                    rsh = rs_pool.tile([128, 2], F32, tag=f"rsh{hi}",
                                       name=f"rsh{hi}")
                    nhalf = 0
                    for half in range(2):
                        # live columns start at the 128-block diagonal edge
                        lo = max(512 * dq + 128 * r128, 1024 * half)
                        hhi = 1024 * (half + 1)
                        if lo >= hhi:
                            continue
                        base = 1024 * half
                        wt = wt_pool.tile([128, 1024], F32, tag="wt",
                                          name="wt")
                        for qc2 in range(max(dq, 2 * half), 2 * half + 2):
                            w0 = 512 * qc2 - base
                            q0 = 512 * qc2
                            if causal and qc2 == dq:
                                # diagonal 128-block: additive mask (resets
                                # PSUM), QK accumulates on top; then plain
                                # QK for the fully-live suffix
                                nc.tensor.matmul(
                                    wt[:, w0 + 128 * r128:w0 + win],
                                    ident_sb[:], mask_tiles[kt][:],
                                    start=True, stop=False)
                                nc.tensor.matmul(
                                    wt[:, w0 + 128 * r128:w0 + win],
                                    kvt_sb[p][plo:phi, ksl],
                                    qt_sb[p][plo:phi,
                                             q0 + 128 * r128:q0 + win],
                                    start=False, stop=True,
                                    tile_position=(plo, 0))
                                if win < 512:
                                    nc.tensor.matmul(
                                        wt[:, w0 + win:w0 + 512],
                                        kvt_sb[p][plo:phi, ksl],
                                        qt_sb[p][plo:phi, q0 + win:q0 + 512],
                                        start=True, stop=True,
                                        tile_position=(plo, 0))
                            else:
                                nc.tensor.matmul(
                                    wt[:, w0:w0 + 512],
                                    kvt_sb[p][plo:phi, ksl],
                                    qt_sb[p][plo:phi, q0:q0 + 512],
                                    start=True, stop=True,
                                    tile_position=(plo, 0))
                        nc.scalar.activation(
                            score[:, lo:hhi], wt[:, lo - base:1024],
                            Exp, accum_out=rsh[:, nhalf:nhalf + 1])
                        nhalf += 1
                    if nhalf == 2:
                        rs = rs_pool.tile([128, 1], F32, tag=f"rs{hi}",
                                          name=f"rs{hi}")
                        nc.vector.tensor_add(rs[:], rsh[:, 0:1], rsh[:, 1:2])
                        rs_ap = rs[:]
                    else:
                        rs_ap = rsh[:, 0:1]
                    ri = rs_pool.tile([128, 1], F32, tag=f"ri{hi}",
                                      name=f"ri{hi}")
                    nc.vector.reciprocal(ri[:], rs_ap)
                    kvs = rs_pool.tile([128, DH], BF16, tag=f"kvs{hi}",
                                       name=f"kvs{hi}")
                    h_local = 2 * p + hi
                    nc.vector.tensor_scalar_mul(
                        kvs[:], kv_sb[kt][:, DH * h_local:DH * h_local + DH],
                        ri[:])
                    heads[hi] = (score, kvs, dq)
                for hi, (plo, phi) in enumerate(((0, 64), (64, 128))):
                    score, kvs, dq = heads[hi]
                    for qc2 in range(dq, QC):
                        q0 = 512 * qc2
                        # on the diagonal k-tile, skip the score columns
                        # left of the 128-block edge (zero / never written);
                        # they were started by earlier k-tiles
                        c0 = q0 + 128 * r128 if (causal and qc2 == dq) else q0
                        stop_kt = 4 * qc2 + 3 if causal else KT - 1
                        nc.tensor.matmul(ot[plo:phi, c0:q0 + 512], kvs[:],
                                         score[:, c0:q0 + 512],
                                         start=(kt == 0),
                                         stop=(kt == stop_kt),
                                         skip_group_check=causal,
                                         tile_position=(0, plo))
            o_sb = o_pool.tile([128, S], BF16, tag="o", name="o")
            nc.vector.tensor_copy(o_sb[:], ot[:])
            nc.sync.dma_start(ag_in[128 * p:128 * p + 128, :], o_sb[:])
            if on_pair is not None:
                on_pair(p)


def build(nc, stage="full", reps=1):
    xTb = nc.dram_tensor("xTb", [D, S], BF16, kind="ExternalInput")
    xT = nc.dram_tensor("xT", [D, S], F32, kind="ExternalInput")
    maskc = nc.dram_tensor("maskc", [S, 128], BF16, kind="ExternalInput")
    ident = nc.dram_tensor("ident", [128, 128], BF16, kind="ExternalInput")
    wq1 = nc.dram_tensor("wq1", [D, HLOC * DH], BF16, kind="ExternalInput")
    wv1 = nc.dram_tensor("wv1", [D, HLOC * DH], BF16, kind="ExternalInput")
    wq2 = nc.dram_tensor("wq2", [D, HLOC * DH], BF16, kind="ExternalInput")
    wv2 = nc.dram_tensor("wv2", [D, HLOC * DH], BF16, kind="ExternalInput")
    w1 = nc.dram_tensor("w1", [D, DFF], BF16, kind="ExternalInput")
    w2 = nc.dram_tensor("w2", [DFF, D], BF16, kind="ExternalInput")
    b1c = nc.dram_tensor("b1c", [DFF, 1], F32, kind="ExternalInput")
    b2c = nc.dram_tensor("b2c", [D, 1], F32, kind="ExternalInput")
    gb1 = nc.dram_tensor("gb1", [2, D], BF16, kind="ExternalInput")
    gb2 = nc.dram_tensor("gb2", [2, D], BF16, kind="ExternalInput")
    gbf = nc.dram_tensor("gbf", [2, D], BF16, kind="ExternalInput")
    cb = nc.dram_tensor("cb", [1, 1], I32, kind="ExternalInput")

    ag1_in = nc.dram_tensor("ag1_in", [NPAIR * 128, S], BF16)
    ag1_outs = [nc.dram_tensor(f"ag1_out{p}", [2 * 128, S], BF16)
                for p in range(NPAIR)]
    x2s = nc.dram_tensor("x2s", [D, S], BF16)
    ag2_in = nc.dram_tensor("ag2_in", [NPAIR * 128, S], BF16)
    ag2_outs = [nc.dram_tensor(f"ag2_out{p}", [2 * 128, S], BF16)
                for p in range(NPAIR)]

    if stage in ("x2", "b1", "b1nm"):
        dbg = nc.dram_tensor("dbg", [D, S], F32, kind="ExternalOutput")
    elif stage == "x3":
        dbg = nc.dram_tensor("dbg", [D, SH], F32, kind="ExternalOutput")
    outT = None
    if stage in ("full", "sim"):
        outT = nc.dram_tensor("outT", [D, SH], F32, kind="ExternalOutput")

    rg = RG

    with tile.TileContext(nc) as tc:
        cv = _scalar_from_input(nc, cb, SH)
        for _rep in range(reps):
            _build_body(tc, nc, stage, cv, locals())


def _all_gather_pair(nc, stage, rg, ag_in, ag_out_p, p):
    """AllGather one head-pair's slice within the 2-core batch group
    (emitted as soon as pair p's o^T is in DRAM, so earlier pairs'
    exchange overlaps later pairs' compute)."""
    in_ap = ag_in[128 * p:128 * p + 128, :]
    if stage.startswith("sim"):
        nc.sync.dma_start(ag_out_p[0:128, :], in_ap)
        nc.sync.dma_start(ag_out_p[128:256, :], in_ap)
    else:
        nc.gpsimd.collective_compute(
            "AllGather", mybir.AluOpType.bypass, replica_groups=rg,
            ins=[in_ap.opt()], outs=[ag_out_p[:].opt()])


def _build_body(tc, nc, stage, cv, env):
    (xTb, xT, maskc, wq1, wv1, wq2, wv2, w1, w2, b1c, b2c, gb1, gb2,
     gbf, x2s, ag1_in, ag1_outs, ag2_in, ag2_outs, rg, ident) = (
        env["xTb"], env["xT"], env["maskc"], env["wq1"], env["wv1"],
        env["wq2"], env["wv2"], env["w1"], env["w2"], env["b1c"],
        env["b2c"], env["gb1"], env["gb2"], env["gbf"], env["x2s"],
        env["ag1_in"], env["ag1_outs"], env["ag2_in"], env["ag2_outs"],
        env["rg"], env["ident"])
    dbg = env.get("dbg")
    outT = env.get("outT")
    with ExitStack() as top:
        const_pool = top.enter_context(tc.tile_pool(name="const", bufs=1))
        ones_b = const_pool.tile([128, 1], BF16, tag="ones_b", name="ones_b")
        nc.vector.memset(ones_b[:], 1.0)
        gb_sb = {}
        for nm, dram in (("gb1", gb1), ("gb2", gb2), ("gbf", gbf)):
            t = const_pool.tile([2, D], BF16, tag=nm, name=nm)
            nc.scalar.dma_start(t[:], dram[:])
            gb_sb[nm] = t
        ident_sb = const_pool.tile([128, 128], BF16, tag="ident", name="ident")
        nc.scalar.dma_start(ident_sb[:], ident[:])

        # ---------------- block 1 ----------------
        # pools that outlive the block-1 scope (stack-ordered before it)
        wpool2 = top.enter_context(tc.tile_pool(name="w2p", bufs=1))
        x2b_pool = top.enter_context(tc.tile_pool(name="x2b", bufs=1))
        with ExitStack() as blk1_outer:
            xb_pool = blk1_outer.enter_context(
                tc.tile_pool(name="xb", bufs=1))
            xb = [xb_pool.tile([128, S], BF16, tag=f"x{dt}", name=f"x{dt}")
                  for dt in range(DT)]
            with ExitStack() as blk1:
                # load order: weights (small, needed first by the dt-outer
                # projection), then x, then mask (needed ~35us later)
                wpool = blk1.enter_context(tc.tile_pool(name="w1p", bufs=1))
                wq_sb, wv_sb = [], []
                for dt in range(DT):
                    wq = wpool.tile([128, HLOC * DH], BF16, tag=f"wq{dt}",
                                    name=f"wq{dt}")
                    nc.sync.dma_start(wq[:], wq1[128 * dt:128 * dt + 128, :])
                    wq_sb.append(wq)
                    wv = wpool.tile([128, HLOC * DH], BF16, tag=f"wv{dt}",
                                    name=f"wv{dt}")
                    nc.sync.dma_start(wv[:], wv1[128 * dt:128 * dt + 128, :])
                    wv_sb.append(wv)
                for dt in range(DT):
                    nc.sync.dma_start(xb[dt][:],
                                      xTb[128 * dt:128 * dt + 128, :])
                xf = [xb_pool.tile([128, S], F32, tag=f"xf{dt}",
                                   name=f"xf{dt}") for dt in range(DT)]
                for dt in range(DT):
                    nc.sync.dma_start(xf[dt][:],
                                      xT[128 * dt:128 * dt + 128, :])
                m_tiles = None
                if stage != "b1nm":
                    mask_pool = blk1.enter_context(
                        tc.tile_pool(name="mask", bufs=1))
                    m_tiles = []
                    for kt in range(KT):
                        m = mask_pool.tile([128, 128], BF16, tag=f"m{kt}",
                                           name=f"m{kt}")
                        nc.sync.dma_start(
                            m[:], maskc[128 * kt:128 * kt + 128, :])
                        m_tiles.append(m)
                _attention(tc, blk1, xb, wq_sb, wv_sb, m_tiles, ag1_in,
                           ident_sb, causal=(stage != "b1nm"),
                           on_pair=lambda p: _all_gather_pair(
                               nc, stage, rg, ag1_in, ag1_outs[p], p))

            if stage in ("b1", "b1nm"):
                with tc.tile_pool(name="b1dbg", bufs=2) as dp:
                    for dt in range(DT):
                        t = dp.tile([128, S], BF16, tag="d", name="d")
                        nc.sync.dma_start(
                            t[:], ag1_outs[dt % NPAIR][
                                128 * (dt // NPAIR):
                                128 * (dt // NPAIR) + 128, :])
                        tf = dp.tile([128, S], F32, tag="df", name="df")
                        nc.vector.tensor_copy(tf[:], t[:])
                        nc.sync.dma_start(dbg[128 * dt:128 * dt + 128, :],
                                          tf[:])
                return

            # ---------------- LN1 -> x2 ----------------
            # prefetch block-2 weights during the gather window
            w2q_sb, w2v_sb = [], []
            for dt in range(DT):
                wq = wpool2.tile([128, HLOC * DH], BF16, tag=f"wq{dt}",
                                 name=f"wq{dt}")
                nc.scalar.dma_start(wq[:], wq2[128 * dt:128 * dt + 128, :])
                w2q_sb.append(wq)
            for dt in range(DT):
                wv = wpool2.tile([128, HLOC * DH], BF16, tag=f"wv{dt}",
                                 name=f"wv{dt}")
                nc.scalar.dma_start(wv[:], wv2[128 * dt:128 * dt + 128, :])
                w2v_sb.append(wv)

            x2b = [x2b_pool.tile([128, S], BF16, tag=f"x2b{dt}",
                                 name=f"x2b{dt}") for dt in range(DT)]
            with ExitStack() as lctx:
                rp = lctx.enter_context(tc.tile_pool(name="ln1_r", bufs=1))
                tp = lctx.enter_context(tc.tile_pool(name="ln1_t", bufs=2))
                r_tiles = [None] * DT
                for i, dt in enumerate(DT_ORDER):
                    t1 = tp.tile([128, S], BF16, tag="ag", name="ag")
                    eng = nc.sync if i % 2 == 0 else nc.scalar
                    eng.dma_start(
                        t1[:], ag1_outs[dt % NPAIR][
                            128 * (dt // NPAIR):128 * (dt // NPAIR) + 128, :])
                    r = rp.tile([128, S], BF16, tag=f"r{dt}", name=f"r{dt}")
                    nc.vector.tensor_add(r[:], t1[:], xf[dt][:])
                    r_tiles[dt] = r
                _layernorm(tc, lctx, r_tiles, gb_sb["gb1"], ones_b, S,
                           None, x2b)

        # spill x2 for the LN2 residual read-back (dynamic column half)
        for dt in range(DT):
            nc.gpsimd.dma_start(x2s[128 * dt:128 * dt + 128, :], x2b[dt][:])

        if stage == "x2":
            with tc.tile_pool(name="x2dbg", bufs=2) as dp:
                for dt in range(DT):
                    tf = dp.tile([128, S], F32, tag="df", name="df")
                    nc.vector.tensor_copy(tf[:], x2b[dt][:])
                    nc.sync.dma_start(dbg[128 * dt:128 * dt + 128, :], tf[:])
            return

        # ---------------- block 2 ----------------
        # prefetch FFN w1 + biases on the Act queue (idle during proj)
        b1_sb, b2_sb, w1_sb = [], [], []
        if stage in ("full", "sim"):
            b_pool = top.enter_context(tc.tile_pool(name="ffn_b", bufs=1))
            w1_pool = top.enter_context(tc.tile_pool(name="ffn_w1", bufs=1))
            for ft in range(FT):
                bt = b_pool.tile([128, 1], F32, tag=f"b1_{ft}",
                                 name=f"b1_{ft}")
                nc.scalar.dma_start(bt[:], b1c[128 * ft:128 * ft + 128, :])
                b1_sb.append(bt)
            for dt in range(DT):
                bt = b_pool.tile([128, 1], F32, tag=f"b2_{dt}",
                                 name=f"b2_{dt}")
                nc.scalar.dma_start(bt[:], b2c[128 * dt:128 * dt + 128, :])
                b2_sb.append(bt)
            for dt in range(DT):
                wt = w1_pool.tile([128, DFF], BF16, tag=f"w1_{dt}",
                                  name=f"w1_{dt}")
                nc.scalar.dma_start(wt[:], w1[128 * dt:128 * dt + 128, :])
                w1_sb.append(wt)

        with ExitStack() as blk2:
            _attention(tc, blk2, x2b, w2q_sb, w2v_sb, None, ag2_in,
                       ident_sb, causal=False,
                       on_pair=lambda p: _all_gather_pair(
                           nc, stage, rg, ag2_in, ag2_outs[p], p),
                       dt_order=DT_ORDER)

        # ---------------- LN2 -> x3 (token half) ----------------
        x3_pool = top.enter_context(tc.tile_pool(name="x3", bufs=1))
        x3b = [x3_pool.tile([128, SH], BF16, tag=f"x3b{dt}", name=f"x3b{dt}")
               for dt in range(DT)]
        with ExitStack() as lctx:
            rp = lctx.enter_context(tc.tile_pool(name="ln2_r", bufs=1))
            tp = lctx.enter_context(tc.tile_pool(name="ln2_t", bufs=2))
            r_tiles = [None] * DT
            for i, dt in enumerate(DT_ORDER):
                t1 = tp.tile([128, SH], BF16, tag="ag", name="ag")
                t2 = tp.tile([128, SH], BF16, tag="xres", name="xres")
                nc.sync.dma_start(
                    t1[:], ag2_outs[dt % NPAIR][
                        128 * (dt // NPAIR):128 * (dt // NPAIR) + 128,
                        bass.ds(cv, SH)])
                nc.scalar.dma_start(
                    t2[:], x2s[128 * dt:128 * dt + 128, bass.ds(cv, SH)])
                r = rp.tile([128, SH], BF16, tag=f"r{dt}", name=f"r{dt}")
                nc.vector.tensor_add(r[:], t1[:], t2[:])
                r_tiles[dt] = r
            _layernorm(tc, lctx, r_tiles, gb_sb["gb2"], ones_b, SH,
                       None, x3b)

        if stage == "x3":
            with tc.tile_pool(name="x3dbg", bufs=2) as dp:
                for dt in range(DT):
                    tf = dp.tile([128, SH], F32, tag="df", name="df")
                    nc.vector.tensor_copy(tf[:], x3b[dt][:])
                    nc.sync.dma_start(dbg[128 * dt:128 * dt + 128, :], tf[:])
            return

        # ---------------- FFN ----------------
        r3_pool = top.enter_context(tc.tile_pool(name="r3", bufs=1))
        r3 = [r3_pool.tile([128, SH], F32, tag=f"r3{dt}", name=f"r3{dt}")
              for dt in range(DT)]
        with ExitStack() as ffn_stack:
            w2_pool = ffn_stack.enter_context(
                tc.tile_pool(name="ffn_w2", bufs=1))
            w2_sb = []
            for ft in range(FT):
                wt = w2_pool.tile([128, D], BF16, tag=f"w2_{ft}",
                                  name=f"w2_{ft}")
                nc.scalar.dma_start(wt[:], w2[128 * ft:128 * ft + 128, :])
                w2_sb.append(wt)
            h_pool = ffn_stack.enter_context(
                tc.tile_pool(name="ffn_h", bufs=3))
            with (
                tc.tile_pool(name="ffn_h_ps", bufs=2, space="PSUM") as hps,
                tc.tile_pool(name="ffn_y_ps", bufs=1, space="PSUM") as yps,
            ):
                for ch in range(SH // 512):
                    cs = slice(512 * ch, 512 * ch + 512)
                    y_ps = [yps.tile([128, 512], F32, tag=f"yp{dt}",
                                     name=f"yp{dt}") for dt in range(DT)]
                    for ft in range(FT):
                        ps = hps.tile([128, 512], F32, tag="hp", name="hp")
                        for i, dt in enumerate(DT_ORDER):
                            nc.tensor.matmul(
                                ps[:], w1_sb[dt][:, 128 * ft:128 * ft + 128],
                                x3b[dt][:, cs],
                                start=(i == 0), stop=(i == DT - 1))
                        h = h_pool.tile([128, 512], BF16, tag="h", name="h")
                        nc.vector.tensor_scalar(h[:], ps[:], b1_sb[ft][:],
                                                0.0, op0=Add, op1=Max)
                        for dt in range(DT):
                            nc.tensor.matmul(
                                y_ps[dt][:],
                                w2_sb[ft][:, 128 * dt:128 * dt + 128],
                                h[:],
                                start=(ft == 0), stop=(ft == FT - 1))
                    for dt in range(DT):
                        nc.vector.scalar_tensor_tensor(
                            r3[dt][:, cs], y_ps[dt][:], b2_sb[dt][:],
                            x3b[dt][:, cs], op0=Add, op1=Add)

        # ---------------- LN3 -> out ----------------
        with ExitStack() as lctx:
            ofin = [r3_pool.tile([128, SH], F32, tag=f"of{dt}",
                                 name=f"of{dt}") for dt in range(DT)]
            _layernorm(tc, lctx, r3, gb_sb["gbf"], ones_b, SH, ofin, None)
            for dt in range(DT):
                nc.sync.dma_start(outT[128 * dt:128 * dt + 128, :],
                                  ofin[dt][:])


_CACHE = {}


def _get_compiled(stage="full"):
    if stage not in _CACHE:
        reps = 1
        name = stage
        import re as _re
        m = _re.match(r"^(.*)_r(\d+)$", stage)
        if m:
            name, reps = m.group(1), int(m.group(2))
        ndev = 1 if name.startswith("sim") else NCORES
        nc = bacc.Bacc("TRN2", target_bir_lowering=False, debug=False,
                       num_devices=ndev)
        build(nc, name, reps=reps)
        nc.compile()
        _CACHE[stage] = nc
    return _CACHE[stage]


def make_in_maps(x, mask, Wq1, Wv1, g1, be1, Wq2, Wv2, g2, be2,
                 Wf1, bf1, Wf2, bf2, gf, bef):
    x = np.asarray(x, np.float32)
    mask = np.asarray(mask)
    maskT = np.where(np.asarray(mask[0, 0]).T, np.float32(-1e9),
                     np.float32(0.0))
    # per-k-tile diagonal 128-block of the additive mask
    maskc = np.empty((S, 128), np.float32)
    for kt in range(KT):
        c0 = 128 * kt
        maskc[128 * kt:128 * kt + 128] = maskT[128 * kt:128 * kt + 128,
                                               c0:c0 + 128]
    maskc = maskc.astype(NP_BF16)
    w1b = np.asarray(Wf1, np.float32).astype(NP_BF16)
    w2b = np.asarray(Wf2, np.float32).astype(NP_BF16)
    scale = np.float32(1.0 / SQRT_DK)
    in_maps = []
    for c in range(NCORES):
        b, hh = c // 2, c % 2
        cols = slice(HLOC * DH * hh, HLOC * DH * (hh + 1))
        xTf = np.ascontiguousarray(x[b].T)
        in_maps.append({
            "xTb": xTf.astype(NP_BF16),
            "xT": xTf,
            "ident": np.eye(128, dtype=np.float32).astype(NP_BF16),
            "maskc": maskc,
            # fold the 1/sqrt(dk) into the Q projection
            "wq1": (np.ascontiguousarray(
                np.asarray(Wq1, np.float32)[:, cols]) * scale).astype(NP_BF16),
            "wv1": np.ascontiguousarray(
                np.asarray(Wv1, np.float32)[:, cols]).astype(NP_BF16),
            "wq2": (np.ascontiguousarray(
                np.asarray(Wq2, np.float32)[:, cols]) * scale).astype(NP_BF16),
            "wv2": np.ascontiguousarray(
                np.asarray(Wv2, np.float32)[:, cols]).astype(NP_BF16),
            "w1": w1b,
            "w2": w2b,
            "b1c": np.asarray(bf1, np.float32).reshape(DFF, 1),
            "b2c": np.asarray(bf2, np.float32).reshape(D, 1),
            "gb1": np.stack([np.asarray(g1, np.float32),
                             np.asarray(be1, np.float32)]).astype(NP_BF16),
            "gb2": np.stack([np.asarray(g2, np.float32),
                             np.asarray(be2, np.float32)]).astype(NP_BF16),
            "gbf": np.stack([np.asarray(gf, np.float32),
                             np.asarray(bef, np.float32)]).astype(NP_BF16),
            "cb": np.array([[SH * hh]], np.int32),
        })
    return in_maps


def run_spmd(in_maps, stage="full"):
    nc = _get_compiled(stage)
    return bass_utils.run_bass_kernel_spmd(nc, in_maps,
                                           core_ids=list(range(NCORES)))


def kernel(**inputs):
    in_maps = make_in_maps(**inputs)
    res = run_spmd(in_maps, "full")
    out = np.empty((B, S, D), np.float32)
    for c in range(NCORES):
        b, hh = c // 2, c % 2
        out[b, SH * hh:SH * (hh + 1), :] = res.results[c]["outT"].T
    return out


class _Runner:
    """Reusable jitted dispatcher (mirrors bass2jax.run_bass_via_pjrt's
    multi-core path) so repeated executions skip re-tracing and host
    transfers — used for timing."""

    def __init__(self, stage="full"):
        import jax
        from jax.sharding import Mesh, PartitionSpec
        from jax.experimental.shard_map import shard_map
        from concourse import bass2jax as b2j

        b2j.install_neuronx_cc_hook()
        nc = _get_compiled(stage)
        pname = (nc.partition_id_tensor.name
                 if nc.partition_id_tensor else None)
        in_names, out_names, out_avals = [], [], []
        for alloc in nc.m.functions[0].allocations:
            if not isinstance(alloc, mybir.MemoryLocationSet):
                continue
            name = alloc.memorylocations[0].name
            if alloc.kind == "ExternalInput":
                if name != pname:
                    in_names.append(name)
            elif alloc.kind == "ExternalOutput":
                out_names.append(name)
                out_avals.append(jax.core.ShapedArray(
                    tuple(alloc.tensor_shape), mybir.dt.np(alloc.dtype)))
        self.in_names, self.out_names = list(in_names), list(out_names)
        self.out_avals = out_avals
        all_in = in_names + out_names
        if pname is not None:
            all_in = all_in + [pname]
        n_params, n_outs = len(in_names), len(out_names)

        def _body(*args):
            operands = list(args)
            if pname is not None:
                operands.append(b2j.partition_id_tensor())
            outs = b2j._bass_exec_p.bind(
                *operands, out_avals=tuple(out_avals), in_names=tuple(all_in),
                out_names=tuple(out_names), lowering_input_output_aliases=(),
                sim_require_finite=True, sim_require_nnan=True, nc=nc)
            return tuple(outs)

        devices = jax.devices()[:NCORES]
        mesh = Mesh(np.asarray(devices), ("core",))
        in_specs = (PartitionSpec("core"),) * (n_params + n_outs)
        out_specs = (PartitionSpec("core"),) * n_outs
        self.fn = jax.jit(
            shard_map(_body, mesh=mesh, in_specs=in_specs,
                      out_specs=out_specs, check_rep=False),
            donate_argnums=tuple(range(n_params, n_params + n_outs)),
            keep_unused=True)
        self._jax = jax

    def device_inputs(self, in_maps):
        import jax
        concat = [np.concatenate([np.asarray(in_maps[c][n])
                                  for c in range(NCORES)], axis=0)
                  for n in self.in_names]
        return [jax.device_put(a) for a in concat]

    def zero_outs(self):
        import jax.numpy as jnp
        return [jnp.zeros((NCORES * av.shape[0], *av.shape[1:]), av.dtype)
                for av in self.out_avals]

    def __call__(self, dev_in, zeros):
        return self.fn(*dev_in, *zeros)


class _RunnerNZ:
    """Timing runner: zero output buffers are created inside the shard_map
    body (device-local), so repeated calls move no host data at all."""

    def __init__(self, stage="full"):
        import jax
        import jax.numpy as jnp
        from jax.sharding import Mesh, PartitionSpec
        from jax.experimental.shard_map import shard_map
        from concourse import bass2jax as b2j

        b2j.install_neuronx_cc_hook()
        nc = _get_compiled(stage)
        pname = (nc.partition_id_tensor.name
                 if nc.partition_id_tensor else None)
        in_names, out_names, out_avals = [], [], []
        for alloc in nc.m.functions[0].allocations:
            if not isinstance(alloc, mybir.MemoryLocationSet):
                continue
            name = alloc.memorylocations[0].name
            if alloc.kind == "ExternalInput":
                if name != pname:
                    in_names.append(name)
            elif alloc.kind == "ExternalOutput":
                out_names.append(name)
                out_avals.append(jax.core.ShapedArray(
                    tuple(alloc.tensor_shape), mybir.dt.np(alloc.dtype)))
        self.in_names, self.out_names = in_names, out_names
        all_in = in_names + out_names
        if pname is not None:
            all_in = all_in + [pname]

        def _body(*args):
            operands = list(args)
            operands += [jnp.zeros(av.shape, av.dtype) for av in out_avals]
            if pname is not None:
                operands.append(b2j.partition_id_tensor())
            outs = b2j._bass_exec_p.bind(
                *operands, out_avals=tuple(out_avals), in_names=tuple(all_in),
                out_names=tuple(out_names), lowering_input_output_aliases=(),
                sim_require_finite=True, sim_require_nnan=True, nc=nc)
            return tuple(outs)

        devices = jax.devices()[:NCORES]
        mesh = Mesh(np.asarray(devices), ("core",))
        self.fn = jax.jit(
            shard_map(_body, mesh=mesh,
                      in_specs=(PartitionSpec("core"),) * len(in_names),
                      out_specs=(PartitionSpec("core"),) * len(out_names),
                      check_rep=False),
            keep_unused=True)

    def device_inputs(self, in_maps):
        import jax
        concat = [np.concatenate([np.asarray(in_maps[c][n])
                                  for c in range(NCORES)], axis=0)
                  for n in self.in_names]
        return [jax.device_put(a) for a in concat]

    def __call__(self, dev_in):
        return self.fn(*dev_in)


# revision 31
# speedup vs baseline: 2.7750x; 2.7750x over previous
"""Trainium2 Bass kernel for a 2-attention-block + FFN decoder stack.

Shapes: x (4, 2048, 768), 12 heads x 64, d_ff 3072.
Sharding over 8 cores: core c handles batch b=c//2 and heads 6*(c%2)..+6 for
both attention blocks; the final FFN+LN runs on token half c%2 of batch b.
Per-pair bf16 AllGathers (replica groups [[0,1],[2,3],...]) exchange the
per-head attention outputs so each core can LayerNorm over the full model
dim.

All compute is done in "transposed" layout (D on partitions, tokens on the
free axis).  The source model's softmax runs over the *query* axis (dim=2
quirk), which in transposed layout (k on partitions, q on free axis) is a
per-partition-row softmax: exp on ScalarE with accum_out produces the row
sums for free; the 1/rowsum is folded into the (tiny) KV matrix instead of
the (huge) score matrix.  No max-subtraction is needed: |w| stays O(10) so
exp cannot overflow, and softmax is shift-invariant.

Block-1 exploits the causal mask: fully-masked 512-wide q-chunks of each
128-row k-tile are skipped entirely (no QK, no exp, no AV -- exp(-1e9)=0
contributes nothing to row sums or AV), and the additive mask matmul runs
only on the diagonal chunk.  kv in token layout comes from PE transposes of
kv^T instead of a second x@Wv matmul.  Projections accumulate with dt as
the outer loop so the preceding LayerNorm's per-tile outputs pipeline
straight into the next block's matmuls.
"""

import os
import sys

for _p in ("/opt/trn_rl_repo", "/root/.axon_site/_ro/trn_rl_repo"):
    if os.path.isdir(_p) and _p not in sys.path:
        sys.path.insert(0, _p)

import numpy as np
from contextlib import ExitStack

from concourse import bass, bacc, mybir, tile
from concourse import bass_utils

F32 = mybir.dt.float32
BF16 = mybir.dt.bfloat16
F8 = mybir.dt.float8e4
I32 = mybir.dt.int32
NP_BF16 = mybir.dt.np(BF16)

B, S, D, H, DH, DFF = 4, 2048, 768, 12, 64, 3072
NCORES = 8
HLOC = 6           # heads per core
NPAIR = 3          # head pairs per core
SQRT_DK = float(np.sqrt(DH))
EPS = 1e-5
SH = S // 2        # token half for FFN
DT = D // 128      # 6 d-tiles
KT = S // 128      # 16 k-tiles
QC = S // 512      # 4 q-chunks
FT = DFF // 128    # 24 ff-tiles

# full 8-rank replica group: 2-core-group collectives are forced onto the
# slow non-Shared path (shared output needs >4 cores), so an 8-rank Shared
# AllGather is faster despite moving 4x the bytes
RG = [list(range(NCORES))]
# LN input processing order: pairs 0,0,1,1,2,2 so the earliest-gathered
# pair's tiles are consumed first
DT_ORDER = [0, 3, 1, 4, 2, 5]

Exp = mybir.ActivationFunctionType.Exp
Sqrt = mybir.ActivationFunctionType.Sqrt
Add = mybir.AluOpType.add
Mult = mybir.AluOpType.mult
Max = mybir.AluOpType.max


def _scalar_from_input(nc, dram, max_val):
    tmp = nc.alloc_registers(f"sv_{dram.name}", mybir.ALL_ENGINES)
    nc.regs_load(tmp, dram[0:1, 0:1])
    return nc.snap(tmp, donate=True, min_val=0, max_val=max_val)


def _layernorm(tc, ctx, r_tiles, gb_sb, ones_b, width, out_f, out_b):
    """LayerNorm over the partition (D) axis of 6 x (128, width) tiles.

    r_tiles may be f32 or bf16; out_f (f32) and out_b (bf16) are optional
    lists of destination tiles.  gb_sb is a (2, 768) bf16 SBUF tile (row 0
    gamma, row 1 beta), applied via tiny outer-product matmuls building
    per-element affine maps.
    """
    nc = tc.nc
    ch_n = width // 512
    sb = ctx.enter_context(tc.tile_pool(name="ln_sb", bufs=1))
    # all row-vector scratch lives at base partition 0 (engine requirement)
    mu = sb.tile([1, width], F32, tag="ln_mu", name="ln_mu")
    msq = sb.tile([1, width], F32, tag="ln_msq", name="ln_msq")
    am = sb.tile([1, width], F32, tag="ln_am", name="ln_am")
    bm = sb.tile([2, width], F32, tag="ln_bm", name="ln_bm")
    nc.vector.memset(bm[0:2, :], 1.0)  # row1 stays ones; row0 overwritten

    with ExitStack() as sctx:
        sq_pool = sctx.enter_context(tc.tile_pool(name="ln_sq", bufs=2))
        ps_pool = sctx.enter_context(
            tc.tile_pool(name="ln_stats_ps", bufs=1, space="PSUM"))
        sum_ps = [ps_pool.tile([1, 512], F32, tag=f"sum{ch}", name=f"sum{ch}")
                  for ch in range(ch_n)]
        ssq_ps = [ps_pool.tile([1, 512], F32, tag=f"ssq{ch}", name=f"ssq{ch}")
                  for ch in range(ch_n)]
        for i, dt in enumerate(DT_ORDER):
            r = r_tiles[dt]
            if r.dtype == BF16:
                rb = r
            else:
                rb = sq_pool.tile([128, width], BF16, tag="rb", name="rb")
                nc.vector.tensor_copy(rb[:], r[:])
            sq = sq_pool.tile([128, width], BF16, tag="sq", name="sq")
            nc.vector.tensor_mul(sq[:], rb[:], rb[:])
            for ch in range(ch_n):
                cs = slice(512 * ch, 512 * ch + 512)
                nc.tensor.matmul(sum_ps[ch][:], ones_b[:, 0:1],
                                 rb[:, cs],
                                 start=(i == 0), stop=(i == DT - 1))
                nc.tensor.matmul(ssq_ps[ch][:], ones_b[:, 0:1],
                                 sq[:, cs],
                                 start=(i == 0), stop=(i == DT - 1))
        for ch in range(ch_n):
            cs = slice(512 * ch, 512 * ch + 512)
            nc.vector.tensor_scalar_mul(mu[0:1, cs], sum_ps[ch][:], 1.0 / D)
            nc.vector.tensor_scalar_mul(msq[0:1, cs], ssq_ps[ch][:], 1.0 / D)

    # var = msq - mu^2 ; sd = sqrt(var + eps) ; rstd = 1/sd ; -mu*rstd
    # chunked so early chunks' broadcast matmuls start before late chunks'
    # stats finish (cuts the serial row-chain latency out of the LN span)
    tmp = sb.tile([1, width], F32, tag="ln_tmp", name="ln_tmp")
    amb = sb.tile([1, width], BF16, tag="ln_amb", name="ln_amb")
    bmb = sb.tile([2, width], BF16, tag="ln_bmb", name="ln_bmb")
    for ch in range(ch_n):
        cs = slice(512 * ch, 512 * ch + 512)
        nc.vector.tensor_mul(tmp[0:1, cs], mu[0:1, cs], mu[0:1, cs])
        nc.vector.tensor_sub(msq[0:1, cs], msq[0:1, cs], tmp[0:1, cs])
        nc.vector.tensor_scalar_add(msq[0:1, cs], msq[0:1, cs], EPS)
        nc.scalar.activation(msq[0:1, cs], msq[0:1, cs], Sqrt)
        nc.vector.reciprocal(am[0:1, cs], msq[0:1, cs])
        nc.vector.scalar_tensor_tensor(bm[0:1, cs], mu[0:1, cs], -1.0,
                                       am[0:1, cs], op0=Mult, op1=Mult)
        nc.vector.tensor_copy(amb[0:1, cs], am[0:1, cs])
        nc.vector.tensor_copy(bmb[0:2, cs], bm[0:2, cs])

    # apply chunk-outer (512 cols of all 6 tiles at a time) so consumers of
    # the first output columns start long before the full apply finishes
    with (
        tc.tile_pool(name="ln_ab_ps", bufs=2, space="PSUM") as ab_pool,
        tc.tile_pool(name="ln_ap", bufs=2) as ap_pool,
    ):
        for ch in range(ch_n):
            cs = slice(512 * ch, 512 * ch + 512)
            for dt in DT_ORDER:
                amat = ab_pool.tile([128, 512], F32, tag="ln_amat",
                                    name="ln_amat")
                bmat = ab_pool.tile([128, 512], F32, tag="ln_bmat",
                                    name="ln_bmat")
                nc.tensor.matmul(amat[:],
                                 gb_sb[0:1, 128 * dt:128 * dt + 128],
                                 amb[0:1, cs], start=True, stop=True)
                nc.tensor.matmul(bmat[:],
                                 gb_sb[0:2, 128 * dt:128 * dt + 128],
                                 bmb[0:2, cs], start=True, stop=True)
                if out_f is not None:
                    dst = out_f[dt]
                    nc.vector.tensor_mul(dst[:, cs], r_tiles[dt][:, cs],
                                         amat[:])
                    nc.vector.tensor_add(dst[:, cs], dst[:, cs], bmat[:])
                    if out_b is not None:
                        nc.vector.tensor_copy(out_b[dt][:, cs],
                                              out_f[dt][:, cs])
                else:
                    # f32 intermediate: only one bf16 rounding on the output
                    tmpa = ap_pool.tile([128, 512], F32, tag="ln_apf",
                                        name="ln_apf")
                    nc.vector.tensor_mul(tmpa[:], r_tiles[dt][:, cs],
                                         amat[:])
                    nc.vector.tensor_add(out_b[dt][:, cs], tmpa[:], bmat[:])


def _attention(tc, ctx, x_tiles, wq_sb, wv_sb, mask_tiles, ag_in, ident_sb,
               causal, on_pair=None, dt_order=None):
    """One attention block in transposed layout (all-bf16 matmul operands).

    x_tiles: 6 x (128, S) bf16 SBUF tiles (caller-owned).
    causal=True skips fully-masked q-regions at 128-column granularity and
    applies mask_tiles (16 x (128, 128) bf16 additive diagonal-block mask)
    via identity-matmul accumulation into the score PSUM.
    dt_order: projection contraction order (to match the order the caller's
    x tiles become ready).
    Writes o^T for this core's 6 heads (384, S) bf16 into ag_in DRAM.
    """
    nc = tc.nc
    if dt_order is None:
        dt_order = list(range(DT))

    qkv_pool = ctx.enter_context(tc.tile_pool(name="attn_qkv", bufs=1))
    kv_pool = ctx.enter_context(tc.tile_pool(name="attn_kv", bufs=1))
    qt_sb = [qkv_pool.tile([128, S], BF16, tag=f"qt{p}", name=f"qt{p}")
             for p in range(NPAIR)]
    kvt_sb = [qkv_pool.tile([128, S], BF16, tag=f"kvt{p}", name=f"kvt{p}")
              for p in range(NPAIR)]
    kv_sb = [kv_pool.tile([128, NPAIR * 128], BF16, tag=f"kv{kt}",
                          name=f"kv{kt}") for kt in range(KT)]

    # projections, dt-outer so x tiles are consumed as they become ready
    with (
        tc.tile_pool(name="attn_proj_ps", bufs=1, space="PSUM") as pps,
        tc.tile_pool(name="attn_tr_ps", bufs=2, space="PSUM") as tps,
    ):
        for qc in range(QC):
            qs = slice(512 * qc, 512 * qc + 512)
            tiles = [pps.tile([128, 512], F32, tag=f"proj{j}",
                              name=f"proj{j}") for j in range(2 * NPAIR)]
            for i, dt in enumerate(dt_order):
                j = 0
                for p in range(NPAIR):
                    for wsb in (wq_sb, wv_sb):
                        nc.tensor.matmul(
                            tiles[j][:], wsb[dt][:, 128 * p:128 * p + 128],
                            x_tiles[dt][:, qs],
                            start=(i == 0), stop=(i == DT - 1))
                        j += 1
            j = 0
            for p in range(NPAIR):
                for dst in (qt_sb, kvt_sb):
                    nc.vector.tensor_copy(dst[p][:, qs], tiles[j][:])
                    j += 1
            # kv token-layout tiles via PE transpose of kv^T
            for kt in range(4 * qc, 4 * qc + 4):
                tp = tps.tile([128, NPAIR * 128], BF16, tag="tr", name="tr")
                for p in range(NPAIR):
                    nc.tensor.matmul(
                        tp[:, 128 * p:128 * p + 128],
                        kvt_sb[p][:, 128 * kt:128 * kt + 128],
                        ident_sb[:], is_transpose=True,
                        start=True, stop=True)
                nc.vector.tensor_copy(kv_sb[kt][:], tp[:])

    # attention proper, one head-pair at a time.
    # PSUM: ot (128,2048)f32 = 4 banks; wt (128,1024)f32 x 2 bufs = 4 banks.
    with (
        tc.tile_pool(name="attn_wt_ps", bufs=2, space="PSUM") as wt_pool,
        tc.tile_pool(name="attn_ot_ps", bufs=1, space="PSUM") as ot_pool,
        tc.tile_pool(name="attn_sc", bufs=3) as sc_pool,
        tc.tile_pool(name="attn_rs", bufs=8) as rs_pool,
        tc.tile_pool(name="attn_o", bufs=2) as o_pool,
    ):
        for p in range(NPAIR):
            ot = ot_pool.tile([128, S], F32, tag="ot", name="ot")
            for kt in range(KT):
                ksl = slice(128 * kt, 128 * kt + 128)
                dq = kt // 4 if causal else 0
                r128 = kt % 4 if causal else 0
                win = 128 * r128 + 128
                heads = {}
                for hi, (plo, phi) in enumerate(((0, 64), (64, 128))):
                    score = sc_pool.tile([128, S], BF16, tag=f"sc{hi}",
                                         name=f"sc{hi}")
                    rsh = rs_pool.tile([128, 2], F32, tag=f"rsh{hi}",
                                       name=f"rsh{hi}")
                    nhalf = 0
                    for half in range(2):
                        # live columns start at the 128-block diagonal edge
                        lo = max(512 * dq + 128 * r128, 1024 * half)
                        hhi = 1024 * (half + 1)
                        if lo >= hhi:
                            continue
                        base = 1024 * half
                        wt = wt_pool.tile([128, 1024], F32, tag="wt",
                                          name="wt")
                        for qc2 in range(max(dq, 2 * half), 2 * half + 2):
                            w0 = 512 * qc2 - base
                            q0 = 512 * qc2
                            if causal and qc2 == dq:
                                # diagonal 128-block: additive mask (resets
                                # PSUM), QK accumulates on top; then plain
                                # QK for the fully-live suffix
                                nc.tensor.matmul(
                                    wt[:, w0 + 128 * r128:w0 + win],
                                    ident_sb[:], mask_tiles[kt][:],
                                    start=True, stop=False)
                                nc.tensor.matmul(
                                    wt[:, w0 + 128 * r128:w0 + win],
                                    kvt_sb[p][plo:phi, ksl],
                                    qt_sb[p][plo:phi,
                                             q0 + 128 * r128:q0 + win],
                                    start=False, stop=True,
                                    tile_position=(plo, 0))
                                if win < 512:
                                    nc.tensor.matmul(
                                        wt[:, w0 + win:w0 + 512],
                                        kvt_sb[p][plo:phi, ksl],
                                        qt_sb[p][plo:phi, q0 + win:q0 + 512],
                                        start=True, stop=True,
                                        tile_position=(plo, 0))
                            else:
                                nc.tensor.matmul(
                                    wt[:, w0:w0 + 512],
                                    kvt_sb[p][plo:phi, ksl],
                                    qt_sb[p][plo:phi, q0:q0 + 512],
                                    start=True, stop=True,
                                    tile_position=(plo, 0))
                        nc.scalar.activation(
                            score[:, lo:hhi], wt[:, lo - base:1024],
                            Exp, accum_out=rsh[:, nhalf:nhalf + 1])
                        nhalf += 1
                    if nhalf == 2:
                        rs = rs_pool.tile([128, 1], F32, tag=f"rs{hi}",
                                          name=f"rs{hi}")
                        nc.vector.tensor_add(rs[:], rsh[:, 0:1], rsh[:, 1:2])
                        rs_ap = rs[:]
                    else:
                        rs_ap = rsh[:, 0:1]
                    ri = rs_pool.tile([128, 1], F32, tag=f"ri{hi}",
                                      name=f"ri{hi}")
                    nc.vector.reciprocal(ri[:], rs_ap)
                    kvs = rs_pool.tile([128, DH], BF16, tag=f"kvs{hi}",
                                       name=f"kvs{hi}")
                    h_local = 2 * p + hi
                    nc.vector.tensor_scalar_mul(
                        kvs[:], kv_sb[kt][:, DH * h_local:DH * h_local + DH],
                        ri[:])
                    heads[hi] = (score, kvs, dq)
                for hi, (plo, phi) in enumerate(((0, 64), (64, 128))):
                    score, kvs, dq = heads[hi]
                    for qc2 in range(dq, QC):
                        q0 = 512 * qc2
                        # on the diagonal k-tile, skip the score columns
                        # left of the 128-block edge (zero / never written);
                        # they were started by earlier k-tiles
                        c0 = q0 + 128 * r128 if (causal and qc2 == dq) else q0
                        stop_kt = 4 * qc2 + 3 if causal else KT - 1
                        nc.tensor.matmul(ot[plo:phi, c0:q0 + 512], kvs[:],
                                         score[:, c0:q0 + 512],
                                         start=(kt == 0),
                                         stop=(kt == stop_kt),
                                         skip_group_check=causal,
                                         tile_position=(0, plo))
            o_sb = o_pool.tile([128, S], BF16, tag="o", name="o")
            nc.vector.tensor_copy(o_sb[:], ot[:])
            nc.sync.dma_start(ag_in[128 * p:128 * p + 128, :], o_sb[:])
            if on_pair is not None:
                on_pair(p)


def build(nc, stage="full", reps=1):
    xTb = nc.dram_tensor("xTb", [D, S], BF16, kind="ExternalInput")
    xT = nc.dram_tensor("xT", [D, S], F32, kind="ExternalInput")
    maskc = nc.dram_tensor("maskc", [S, 128], BF16, kind="ExternalInput")
    ident = nc.dram_tensor("ident", [128, 128], BF16, kind="ExternalInput")
    wq1 = nc.dram_tensor("wq1", [D, HLOC * DH], BF16, kind="ExternalInput")
    wv1 = nc.dram_tensor("wv1", [D, HLOC * DH], BF16, kind="ExternalInput")
    wq2 = nc.dram_tensor("wq2", [D, HLOC * DH], BF16, kind="ExternalInput")
    wv2 = nc.dram_tensor("wv2", [D, HLOC * DH], BF16, kind="ExternalInput")
    w1 = nc.dram_tensor("w1", [D, DFF], BF16, kind="ExternalInput")
    w2 = nc.dram_tensor("w2", [DFF, D], BF16, kind="ExternalInput")
    b1c = nc.dram_tensor("b1c", [DFF, 1], F32, kind="ExternalInput")
    b2c = nc.dram_tensor("b2c", [D, 1], F32, kind="ExternalInput")
    gb1 = nc.dram_tensor("gb1", [2, D], BF16, kind="ExternalInput")
    gb2 = nc.dram_tensor("gb2", [2, D], BF16, kind="ExternalInput")
    gbf = nc.dram_tensor("gbf", [2, D], BF16, kind="ExternalInput")
    cb = nc.dram_tensor("cb", [1, 1], I32, kind="ExternalInput")
    rb = nc.dram_tensor("rb", [1, 1], I32, kind="ExternalInput")

    ag1_in = nc.dram_tensor("ag1_in", [NPAIR * 128, S], BF16)
    ag1_outs = [nc.dram_tensor(f"ag1_out{p}", [NCORES * 128, S], BF16,
                               addr_space="Shared") for p in range(NPAIR)]
    x2s = nc.dram_tensor("x2s", [D, S], BF16)
    ag2_in = nc.dram_tensor("ag2_in", [NPAIR * 128, S], BF16)
    ag2_outs = [nc.dram_tensor(f"ag2_out{p}", [NCORES * 128, S], BF16,
                               addr_space="Shared") for p in range(NPAIR)]

    if stage in ("x2", "b1", "b1nm"):
        dbg = nc.dram_tensor("dbg", [D, S], F32, kind="ExternalOutput")
    elif stage == "x3":
        dbg = nc.dram_tensor("dbg", [D, SH], F32, kind="ExternalOutput")
    outT = None
    if stage in ("full", "sim", "fullnc"):
        outT = nc.dram_tensor("outT", [D, SH], F32, kind="ExternalOutput")

    rg = RG

    with tile.TileContext(nc) as tc:
        cv = _scalar_from_input(nc, cb, SH)
        rv = _scalar_from_input(nc, rb, 256 * (B - 1))
        for _rep in range(reps):
            _build_body(tc, nc, stage, cv, rv, locals())


def _all_gather_pair(nc, stage, rg, ag_in, ag_out_p, p):
    """AllGather one head-pair's slice within the 2-core batch group
    (emitted as soon as pair p's o^T is in DRAM, so earlier pairs'
    exchange overlaps later pairs' compute)."""
    in_ap = ag_in[128 * p:128 * p + 128, :]
    if stage.startswith("sim") or stage.startswith("fullnc"):
        nc.sync.dma_start(ag_out_p[0:128, :], in_ap)
        nc.sync.dma_start(ag_out_p[128:256, :], in_ap)
    else:
        nc.gpsimd.collective_compute(
            "AllGather", mybir.AluOpType.bypass, replica_groups=rg,
            ins=[in_ap.opt()], outs=[ag_out_p[:].opt()])


def _build_body(tc, nc, stage, cv, rv, env):
    (xTb, xT, maskc, wq1, wv1, wq2, wv2, w1, w2, b1c, b2c, gb1, gb2,
     gbf, x2s, ag1_in, ag1_outs, ag2_in, ag2_outs, rg, ident) = (
        env["xTb"], env["xT"], env["maskc"], env["wq1"], env["wv1"],
        env["wq2"], env["wv2"], env["w1"], env["w2"], env["b1c"],
        env["b2c"], env["gb1"], env["gb2"], env["gbf"], env["x2s"],
        env["ag1_in"], env["ag1_outs"], env["ag2_in"], env["ag2_outs"],
        env["rg"], env["ident"])
    dbg = env.get("dbg")
    outT = env.get("outT")
    with ExitStack() as top:
        const_pool = top.enter_context(tc.tile_pool(name="const", bufs=1))
        ones_b = const_pool.tile([128, 1], BF16, tag="ones_b", name="ones_b")
        nc.vector.memset(ones_b[:], 1.0)
        gb_sb = {}
        for nm, dram in (("gb1", gb1), ("gb2", gb2), ("gbf", gbf)):
            t = const_pool.tile([2, D], BF16, tag=nm, name=nm)
            nc.scalar.dma_start(t[:], dram[:])
            gb_sb[nm] = t
        ident_sb = const_pool.tile([128, 128], BF16, tag="ident", name="ident")
        nc.scalar.dma_start(ident_sb[:], ident[:])

        # ---------------- block 1 ----------------
        # pools that outlive the block-1 scope (stack-ordered before it)
        wpool2 = top.enter_context(tc.tile_pool(name="w2p", bufs=1))
        x2b_pool = top.enter_context(tc.tile_pool(name="x2b", bufs=1))
        with ExitStack() as blk1_outer:
            xb_pool = blk1_outer.enter_context(
                tc.tile_pool(name="xb", bufs=1))
            xb = [xb_pool.tile([128, S], BF16, tag=f"x{dt}", name=f"x{dt}")
                  for dt in range(DT)]
            with ExitStack() as blk1:
                # load order: weights (small, needed first by the dt-outer
                # projection), then x, then mask (needed ~35us later)
                wpool = blk1.enter_context(tc.tile_pool(name="w1p", bufs=1))
                wq_sb, wv_sb = [], []
                for dt in range(DT):
                    wq = wpool.tile([128, HLOC * DH], BF16, tag=f"wq{dt}",
                                    name=f"wq{dt}")
                    nc.sync.dma_start(wq[:], wq1[128 * dt:128 * dt + 128, :])
                    wq_sb.append(wq)
                    wv = wpool.tile([128, HLOC * DH], BF16, tag=f"wv{dt}",
                                    name=f"wv{dt}")
                    nc.sync.dma_start(wv[:], wv1[128 * dt:128 * dt + 128, :])
                    wv_sb.append(wv)
                    nc.sync.dma_start(xb[dt][:],
                                      xTb[128 * dt:128 * dt + 128, :])
                xf = [xb_pool.tile([128, S], F32, tag=f"xf{dt}",
                                   name=f"xf{dt}") for dt in range(DT)]
                for dt in range(DT):
                    nc.sync.dma_start(xf[dt][:],
                                      xT[128 * dt:128 * dt + 128, :])
                m_tiles = None
                if stage != "b1nm":
                    mask_pool = blk1.enter_context(
                        tc.tile_pool(name="mask", bufs=1))
                    m_tiles = []
                    for kt in range(KT):
                        m = mask_pool.tile([128, 128], BF16, tag=f"m{kt}",
                                           name=f"m{kt}")
                        nc.sync.dma_start(
                            m[:], maskc[128 * kt:128 * kt + 128, :])
                        m_tiles.append(m)
                _attention(tc, blk1, xb, wq_sb, wv_sb, m_tiles, ag1_in,
                           ident_sb, causal=(stage != "b1nm"),
                           on_pair=lambda p: _all_gather_pair(
                               nc, stage, rg, ag1_in, ag1_outs[p], p))

            if stage in ("b1", "b1nm"):
                with tc.tile_pool(name="b1dbg", bufs=2) as dp:
                    for dt in range(DT):
                        t = dp.tile([128, S], BF16, tag="d", name="d")
                        nc.sync.dma_start(
                            t[:], ag1_outs[dt % NPAIR][
                                bass.ds(rv + 128 * (dt // NPAIR), 128), :])
                        tf = dp.tile([128, S], F32, tag="df", name="df")
                        nc.vector.tensor_copy(tf[:], t[:])
                        nc.sync.dma_start(dbg[128 * dt:128 * dt + 128, :],
                                          tf[:])
                return

            # ---------------- LN1 -> x2 ----------------
            # prefetch block-2 weights during the gather window
            w2q_sb, w2v_sb = [], []
            for dt in range(DT):
                wq = wpool2.tile([128, HLOC * DH], BF16, tag=f"wq{dt}",
                                 name=f"wq{dt}")
                nc.scalar.dma_start(wq[:], wq2[128 * dt:128 * dt + 128, :])
                w2q_sb.append(wq)
            for dt in range(DT):
                wv = wpool2.tile([128, HLOC * DH], BF16, tag=f"wv{dt}",
                                 name=f"wv{dt}")
                nc.scalar.dma_start(wv[:], wv2[128 * dt:128 * dt + 128, :])
                w2v_sb.append(wv)

            x2b = [x2b_pool.tile([128, S], BF16, tag=f"x2b{dt}",
                                 name=f"x2b{dt}") for dt in range(DT)]
            with ExitStack() as lctx:
                rp = lctx.enter_context(tc.tile_pool(name="ln1_r", bufs=1))
                tp = lctx.enter_context(tc.tile_pool(name="ln1_t", bufs=2))
                r_tiles = [None] * DT
                for i, dt in enumerate(DT_ORDER):
                    t1 = tp.tile([128, S], BF16, tag="ag", name="ag")
                    eng = nc.sync if i % 2 == 0 else nc.scalar
                    eng.dma_start(
                        t1[:], ag1_outs[dt % NPAIR][
                            bass.ds(rv + 128 * (dt // NPAIR), 128), :])
                    r = rp.tile([128, S], BF16, tag=f"r{dt}", name=f"r{dt}")
                    nc.vector.tensor_add(r[:], t1[:], xf[dt][:])
                    r_tiles[dt] = r
                _layernorm(tc, lctx, r_tiles, gb_sb["gb1"], ones_b, S,
                           None, x2b)

        # spill x2 for the LN2 residual read-back (dynamic column half)
        for dt in range(DT):
            nc.sync.dma_start(x2s[128 * dt:128 * dt + 128, :], x2b[dt][:])

        if stage == "x2":
            with tc.tile_pool(name="x2dbg", bufs=2) as dp:
                for dt in range(DT):
                    tf = dp.tile([128, S], F32, tag="df", name="df")
                    nc.vector.tensor_copy(tf[:], x2b[dt][:])
                    nc.sync.dma_start(dbg[128 * dt:128 * dt + 128, :], tf[:])
            return

        # ---------------- block 2 ----------------
        # prefetch FFN w1 + biases on the Act queue (idle during proj)
        b1_sb, b2_sb, w1_sb = [], [], []
        if stage in ("full", "sim", "fullnc"):
            b_pool = top.enter_context(tc.tile_pool(name="ffn_b", bufs=1))
            w1_pool = top.enter_context(tc.tile_pool(name="ffn_w1", bufs=1))
            for ft in range(FT):
                bt = b_pool.tile([128, 1], F32, tag=f"b1_{ft}",
                                 name=f"b1_{ft}")
                nc.scalar.dma_start(bt[:], b1c[128 * ft:128 * ft + 128, :])
                b1_sb.append(bt)
            for dt in range(DT):
                bt = b_pool.tile([128, 1], F32, tag=f"b2_{dt}",
                                 name=f"b2_{dt}")
                nc.scalar.dma_start(bt[:], b2c[128 * dt:128 * dt + 128, :])
                b2_sb.append(bt)
            for dt in range(DT):
                wt = w1_pool.tile([128, DFF], BF16, tag=f"w1_{dt}",
                                  name=f"w1_{dt}")
                nc.scalar.dma_start(wt[:], w1[128 * dt:128 * dt + 128, :])
                w1_sb.append(wt)

        with ExitStack() as blk2:
            _attention(tc, blk2, x2b, w2q_sb, w2v_sb, None, ag2_in,
                       ident_sb, causal=False,
                       on_pair=lambda p: _all_gather_pair(
                           nc, stage, rg, ag2_in, ag2_outs[p], p),
                       dt_order=DT_ORDER)

        # ---------------- LN2 -> x3 (token half) ----------------
        x3_pool = top.enter_context(tc.tile_pool(name="x3", bufs=1))
        x3b = [x3_pool.tile([128, SH], BF16, tag=f"x3b{dt}", name=f"x3b{dt}")
               for dt in range(DT)]
        with ExitStack() as lctx:
            rp = lctx.enter_context(tc.tile_pool(name="ln2_r", bufs=1))
            tp = lctx.enter_context(tc.tile_pool(name="ln2_t", bufs=2))
            r_tiles = [None] * DT
            for i, dt in enumerate(DT_ORDER):
                t1 = tp.tile([128, SH], BF16, tag="ag", name="ag")
                t2 = tp.tile([128, SH], BF16, tag="xres", name="xres")
                nc.sync.dma_start(
                    t1[:], ag2_outs[dt % NPAIR][
                        bass.ds(rv + 128 * (dt // NPAIR), 128),
                        bass.ds(cv, SH)])
                nc.scalar.dma_start(
                    t2[:], x2s[128 * dt:128 * dt + 128, bass.ds(cv, SH)])
                r = rp.tile([128, SH], BF16, tag=f"r{dt}", name=f"r{dt}")
                nc.vector.tensor_add(r[:], t1[:], t2[:])
                r_tiles[dt] = r
            _layernorm(tc, lctx, r_tiles, gb_sb["gb2"], ones_b, SH,
                       None, x3b)

        if stage == "x3":
            with tc.tile_pool(name="x3dbg", bufs=2) as dp:
                for dt in range(DT):
                    tf = dp.tile([128, SH], F32, tag="df", name="df")
                    nc.vector.tensor_copy(tf[:], x3b[dt][:])
                    nc.sync.dma_start(dbg[128 * dt:128 * dt + 128, :], tf[:])
            return

        # ---------------- FFN ----------------
        r3_pool = top.enter_context(tc.tile_pool(name="r3", bufs=1))
        r3 = [r3_pool.tile([128, SH], F32, tag=f"r3{dt}", name=f"r3{dt}")
              for dt in range(DT)]
        with ExitStack() as ffn_stack:
            w2_pool = ffn_stack.enter_context(
                tc.tile_pool(name="ffn_w2", bufs=1))
            w2_sb = []
            for ft in range(FT):
                wt = w2_pool.tile([128, D], BF16, tag=f"w2_{ft}",
                                  name=f"w2_{ft}")
                nc.scalar.dma_start(wt[:], w2[128 * ft:128 * ft + 128, :])
                w2_sb.append(wt)
            h_pool = ffn_stack.enter_context(
                tc.tile_pool(name="ffn_h", bufs=3))
            with (
                tc.tile_pool(name="ffn_h_ps", bufs=2, space="PSUM") as hps,
                tc.tile_pool(name="ffn_y_ps", bufs=1, space="PSUM") as yps,
            ):
                for ch in range(SH // 512):
                    cs = slice(512 * ch, 512 * ch + 512)
                    y_ps = [yps.tile([128, 512], F32, tag=f"yp{dt}",
                                     name=f"yp{dt}") for dt in range(DT)]
                    for ft in range(FT):
                        ps = hps.tile([128, 512], F32, tag="hp", name="hp")
                        for i, dt in enumerate(DT_ORDER):
                            nc.tensor.matmul(
                                ps[:], w1_sb[dt][:, 128 * ft:128 * ft + 128],
                                x3b[dt][:, cs],
                                start=(i == 0), stop=(i == DT - 1))
                        h = h_pool.tile([128, 512], BF16, tag="h", name="h")
                        nc.vector.tensor_scalar(h[:], ps[:], b1_sb[ft][:],
                                                0.0, op0=Add, op1=Max)
                        for dt in range(DT):
                            nc.tensor.matmul(
                                y_ps[dt][:],
                                w2_sb[ft][:, 128 * dt:128 * dt + 128],
                                h[:],
                                start=(ft == 0), stop=(ft == FT - 1))
                    for dt in range(DT):
                        nc.vector.scalar_tensor_tensor(
                            r3[dt][:, cs], y_ps[dt][:], b2_sb[dt][:],
                            x3b[dt][:, cs], op0=Add, op1=Add)

        # ---------------- LN3 -> out ----------------
        with ExitStack() as lctx:
            ofin = [r3_pool.tile([128, SH], F32, tag=f"of{dt}",
                                 name=f"of{dt}") for dt in range(DT)]
            _layernorm(tc, lctx, r3, gb_sb["gbf"], ones_b, SH, ofin, None)
            for dt in range(DT):
                for ch in range(SH // 512):
                    cs = slice(512 * ch, 512 * ch + 512)
                    nc.sync.dma_start(outT[128 * dt:128 * dt + 128, cs],
                                      ofin[dt][:, cs])


_CACHE = {}


def _get_compiled(stage="full"):
    if stage not in _CACHE:
        reps = 1
        name = stage
        import re as _re
        m = _re.match(r"^(.*)_r(\d+)$", stage)
        if m:
            name, reps = m.group(1), int(m.group(2))
        ndev = 1 if name.startswith("sim") else NCORES
        nc = bacc.Bacc("TRN2", target_bir_lowering=False, debug=False,
                       num_devices=ndev)
        build(nc, name, reps=reps)
        nc.compile()
        _CACHE[stage] = nc
    return _CACHE[stage]


def make_in_maps(x, mask, Wq1, Wv1, g1, be1, Wq2, Wv2, g2, be2,
                 Wf1, bf1, Wf2, bf2, gf, bef):
    x = np.asarray(x, np.float32)
    mask = np.asarray(mask)
    maskT = np.where(np.asarray(mask[0, 0]).T, np.float32(-1e9),
                     np.float32(0.0))
    # per-k-tile diagonal 128-block of the additive mask
    maskc = np.empty((S, 128), np.float32)
    for kt in range(KT):
        c0 = 128 * kt
        maskc[128 * kt:128 * kt + 128] = maskT[128 * kt:128 * kt + 128,
                                               c0:c0 + 128]
    maskc = maskc.astype(NP_BF16)
    w1b = np.asarray(Wf1, np.float32).astype(NP_BF16)
    w2b = np.asarray(Wf2, np.float32).astype(NP_BF16)
    scale = np.float32(1.0 / SQRT_DK)
    in_maps = []
    for c in range(NCORES):
        b, hh = c // 2, c % 2
        cols = slice(HLOC * DH * hh, HLOC * DH * (hh + 1))
        xTf = np.ascontiguousarray(x[b].T)
        in_maps.append({
            "xTb": xTf.astype(NP_BF16),
            "xT": xTf,
            "ident": np.eye(128, dtype=np.float32).astype(NP_BF16),
            "maskc": maskc,
            # fold the 1/sqrt(dk) into the Q projection
            "wq1": (np.ascontiguousarray(
                np.asarray(Wq1, np.float32)[:, cols]) * scale).astype(NP_BF16),
            "wv1": np.ascontiguousarray(
                np.asarray(Wv1, np.float32)[:, cols]).astype(NP_BF16),
            "wq2": (np.ascontiguousarray(
                np.asarray(Wq2, np.float32)[:, cols]) * scale).astype(NP_BF16),
            "wv2": np.ascontiguousarray(
                np.asarray(Wv2, np.float32)[:, cols]).astype(NP_BF16),
            "w1": w1b,
            "w2": w2b,
            "b1c": np.asarray(bf1, np.float32).reshape(DFF, 1),
            "b2c": np.asarray(bf2, np.float32).reshape(D, 1),
            "gb1": np.stack([np.asarray(g1, np.float32),
                             np.asarray(be1, np.float32)]).astype(NP_BF16),
            "gb2": np.stack([np.asarray(g2, np.float32),
                             np.asarray(be2, np.float32)]).astype(NP_BF16),
            "gbf": np.stack([np.asarray(gf, np.float32),
                             np.asarray(bef, np.float32)]).astype(NP_BF16),
            "cb": np.array([[SH * hh]], np.int32),
            "rb": np.array([[256 * b]], np.int32),
        })
    return in_maps


def run_spmd(in_maps, stage="full"):
    nc = _get_compiled(stage)
    return bass_utils.run_bass_kernel_spmd(nc, in_maps,
                                           core_ids=list(range(NCORES)))


def kernel(**inputs):
    in_maps = make_in_maps(**inputs)
    res = run_spmd(in_maps, "full")
    out = np.empty((B, S, D), np.float32)
    for c in range(NCORES):
        b, hh = c // 2, c % 2
        out[b, SH * hh:SH * (hh + 1), :] = res.results[c]["outT"].T
    return out


class _Runner:
    """Reusable jitted dispatcher (mirrors bass2jax.run_bass_via_pjrt's
    multi-core path) so repeated executions skip re-tracing and host
    transfers — used for timing."""

    def __init__(self, stage="full"):
        import jax
        from jax.sharding import Mesh, PartitionSpec
        from jax.experimental.shard_map import shard_map
        from concourse import bass2jax as b2j

        b2j.install_neuronx_cc_hook()
        nc = _get_compiled(stage)
        pname = (nc.partition_id_tensor.name
                 if nc.partition_id_tensor else None)
        in_names, out_names, out_avals = [], [], []
        for alloc in nc.m.functions[0].allocations:
            if not isinstance(alloc, mybir.MemoryLocationSet):
                continue
            name = alloc.memorylocations[0].name
            if alloc.kind == "ExternalInput":
                if name != pname:
                    in_names.append(name)
            elif alloc.kind == "ExternalOutput":
                out_names.append(name)
                out_avals.append(jax.core.ShapedArray(
                    tuple(alloc.tensor_shape), mybir.dt.np(alloc.dtype)))
        self.in_names, self.out_names = list(in_names), list(out_names)
        self.out_avals = out_avals
        all_in = in_names + out_names
        if pname is not None:
            all_in = all_in + [pname]
        n_params, n_outs = len(in_names), len(out_names)

        def _body(*args):
            operands = list(args)
            if pname is not None:
                operands.append(b2j.partition_id_tensor())
            outs = b2j._bass_exec_p.bind(
                *operands, out_avals=tuple(out_avals), in_names=tuple(all_in),
                out_names=tuple(out_names), lowering_input_output_aliases=(),
                sim_require_finite=True, sim_require_nnan=True, nc=nc)
            return tuple(outs)

        devices = jax.devices()[:NCORES]
        mesh = Mesh(np.asarray(devices), ("core",))
        in_specs = (PartitionSpec("core"),) * (n_params + n_outs)
        out_specs = (PartitionSpec("core"),) * n_outs
        self.fn = jax.jit(
            shard_map(_body, mesh=mesh, in_specs=in_specs,
                      out_specs=out_specs, check_rep=False),
            donate_argnums=tuple(range(n_params, n_params + n_outs)),
            keep_unused=True)
        self._jax = jax

    def device_inputs(self, in_maps):
        import jax
        concat = [np.concatenate([np.asarray(in_maps[c][n])
                                  for c in range(NCORES)], axis=0)
                  for n in self.in_names]
        return [jax.device_put(a) for a in concat]

    def zero_outs(self):
        import jax.numpy as jnp
        return [jnp.zeros((NCORES * av.shape[0], *av.shape[1:]), av.dtype)
                for av in self.out_avals]

    def __call__(self, dev_in, zeros):
        return self.fn(*dev_in, *zeros)


class _RunnerNZ:
    """Timing runner: zero output buffers are created inside the shard_map
    body (device-local), so repeated calls move no host data at all."""

    def __init__(self, stage="full"):
        import jax
        import jax.numpy as jnp
        from jax.sharding import Mesh, PartitionSpec
        from jax.experimental.shard_map import shard_map
        from concourse import bass2jax as b2j

        b2j.install_neuronx_cc_hook()
        nc = _get_compiled(stage)
        pname = (nc.partition_id_tensor.name
                 if nc.partition_id_tensor else None)
        in_names, out_names, out_avals = [], [], []
        for alloc in nc.m.functions[0].allocations:
            if not isinstance(alloc, mybir.MemoryLocationSet):
                continue
            name = alloc.memorylocations[0].name
            if alloc.kind == "ExternalInput":
                if name != pname:
                    in_names.append(name)
            elif alloc.kind == "ExternalOutput":
                out_names.append(name)
                out_avals.append(jax.core.ShapedArray(
                    tuple(alloc.tensor_shape), mybir.dt.np(alloc.dtype)))
        self.in_names, self.out_names = in_names, out_names
        all_in = in_names + out_names
        if pname is not None:
            all_in = all_in + [pname]

        def _body(*args):
            operands = list(args)
            operands += [jnp.zeros(av.shape, av.dtype) for av in out_avals]
            if pname is not None:
                operands.append(b2j.partition_id_tensor())
            outs = b2j._bass_exec_p.bind(
                *operands, out_avals=tuple(out_avals), in_names=tuple(all_in),
                out_names=tuple(out_names), lowering_input_output_aliases=(),
                sim_require_finite=True, sim_require_nnan=True, nc=nc)
            return tuple(outs)

        devices = jax.devices()[:NCORES]
        mesh = Mesh(np.asarray(devices), ("core",))
        self.fn = jax.jit(
            shard_map(_body, mesh=mesh,
                      in_specs=(PartitionSpec("core"),) * len(in_names),
                      out_specs=(PartitionSpec("core"),) * len(out_names),
                      check_rep=False),
            keep_unused=True)

    def device_inputs(self, in_maps):
        import jax
        concat = [np.concatenate([np.asarray(in_maps[c][n])
                                  for c in range(NCORES)], axis=0)
                  for n in self.in_names]
        return [jax.device_put(a) for a in concat]

    def __call__(self, dev_in):
        return self.fn(*dev_in)


# revision 33
# speedup vs baseline: 2.8077x; 1.0118x over previous
"""Trainium2 Bass kernel for a 2-attention-block + FFN decoder stack.

Shapes: x (4, 2048, 768), 12 heads x 64, d_ff 3072.
Sharding over 8 cores: core c handles batch b=c//2 and heads 6*(c%2)..+6 for
both attention blocks; the final FFN+LN runs on token half c%2 of batch b.
Per-pair bf16 AllGathers (replica groups [[0,1],[2,3],...]) exchange the
per-head attention outputs so each core can LayerNorm over the full model
dim.

All compute is done in "transposed" layout (D on partitions, tokens on the
free axis).  The source model's softmax runs over the *query* axis (dim=2
quirk), which in transposed layout (k on partitions, q on free axis) is a
per-partition-row softmax: exp on ScalarE with accum_out produces the row
sums for free; the 1/rowsum is folded into the (tiny) KV matrix instead of
the (huge) score matrix.  No max-subtraction is needed: |w| stays O(10) so
exp cannot overflow, and softmax is shift-invariant.

Block-1 exploits the causal mask: fully-masked 512-wide q-chunks of each
128-row k-tile are skipped entirely (no QK, no exp, no AV -- exp(-1e9)=0
contributes nothing to row sums or AV), and the additive mask matmul runs
only on the diagonal chunk.  kv in token layout comes from PE transposes of
kv^T instead of a second x@Wv matmul.  Projections accumulate with dt as
the outer loop so the preceding LayerNorm's per-tile outputs pipeline
straight into the next block's matmuls.
"""

import os
import sys

for _p in ("/opt/trn_rl_repo", "/root/.axon_site/_ro/trn_rl_repo"):
    if os.path.isdir(_p) and _p not in sys.path:
        sys.path.insert(0, _p)

import numpy as np
from contextlib import ExitStack

from concourse import bass, bacc, mybir, tile
from concourse import bass_utils

F32 = mybir.dt.float32
BF16 = mybir.dt.bfloat16
F8 = mybir.dt.float8e4
I32 = mybir.dt.int32
NP_BF16 = mybir.dt.np(BF16)

B, S, D, H, DH, DFF = 4, 2048, 768, 12, 64, 3072
NCORES = 8
HLOC = 6           # heads per core
NPAIR = 3          # head pairs per core
SQRT_DK = float(np.sqrt(DH))
EPS = 1e-5
SH = S // 2        # token half for FFN
DT = D // 128      # 6 d-tiles
KT = S // 128      # 16 k-tiles
QC = S // 512      # 4 q-chunks
FT = DFF // 128    # 24 ff-tiles

# full 8-rank replica group: 2-core-group collectives are forced onto the
# slow non-Shared path (shared output needs >4 cores), so an 8-rank Shared
# AllGather is faster despite moving 4x the bytes
RG = [list(range(NCORES))]
# LN input processing order: pairs 0,0,1,1,2,2 so the earliest-gathered
# pair's tiles are consumed first
DT_ORDER = [0, 3, 1, 4, 2, 5]

Exp = mybir.ActivationFunctionType.Exp
Sqrt = mybir.ActivationFunctionType.Sqrt
Add = mybir.AluOpType.add
Mult = mybir.AluOpType.mult
Max = mybir.AluOpType.max


def _scalar_from_input(nc, dram, max_val):
    tmp = nc.alloc_registers(f"sv_{dram.name}", mybir.ALL_ENGINES)
    nc.regs_load(tmp, dram[0:1, 0:1])
    return nc.snap(tmp, donate=True, min_val=0, max_val=max_val)


def _layernorm(tc, ctx, r_tiles, gb_sb, ones_b, width, out_f, out_b):
    """LayerNorm over the partition (D) axis of 6 x (128, width) tiles.

    r_tiles may be f32 or bf16; out_f (f32) and out_b (bf16) are optional
    lists of destination tiles.  gb_sb is a (2, 768) bf16 SBUF tile (row 0
    gamma, row 1 beta), applied via tiny outer-product matmuls building
    per-element affine maps.
    """
    nc = tc.nc
    ch_n = width // 512
    sb = ctx.enter_context(tc.tile_pool(name="ln_sb", bufs=1))
    # all row-vector scratch lives at base partition 0 (engine requirement)
    mu = sb.tile([1, width], F32, tag="ln_mu", name="ln_mu")
    msq = sb.tile([1, width], F32, tag="ln_msq", name="ln_msq")
    am = sb.tile([1, width], F32, tag="ln_am", name="ln_am")
    bm = sb.tile([2, width], F32, tag="ln_bm", name="ln_bm")
    nc.vector.memset(bm[0:2, :], 1.0)  # row1 stays ones; row0 overwritten

    with ExitStack() as sctx:
        sq_pool = sctx.enter_context(tc.tile_pool(name="ln_sq", bufs=2))
        ps_pool = sctx.enter_context(
            tc.tile_pool(name="ln_stats_ps", bufs=1, space="PSUM"))
        sum_ps = [ps_pool.tile([1, 512], F32, tag=f"sum{ch}", name=f"sum{ch}")
                  for ch in range(ch_n)]
        ssq_ps = [ps_pool.tile([1, 512], F32, tag=f"ssq{ch}", name=f"ssq{ch}")
                  for ch in range(ch_n)]
        for i, dt in enumerate(DT_ORDER):
            r = r_tiles[dt]
            if r.dtype == BF16:
                rb = r
            else:
                rb = sq_pool.tile([128, width], BF16, tag="rb", name="rb")
                nc.vector.tensor_copy(rb[:], r[:])
            sq = sq_pool.tile([128, width], BF16, tag="sq", name="sq")
            nc.vector.tensor_mul(sq[:], rb[:], rb[:])
            for ch in range(ch_n):
                cs = slice(512 * ch, 512 * ch + 512)
                nc.tensor.matmul(sum_ps[ch][:], ones_b[:, 0:1],
                                 rb[:, cs],
                                 start=(i == 0), stop=(i == DT - 1))
                nc.tensor.matmul(ssq_ps[ch][:], ones_b[:, 0:1],
                                 sq[:, cs],
                                 start=(i == 0), stop=(i == DT - 1))
        for ch in range(ch_n):
            cs = slice(512 * ch, 512 * ch + 512)
            nc.vector.tensor_scalar_mul(mu[0:1, cs], sum_ps[ch][:], 1.0 / D)
            nc.vector.tensor_scalar_mul(msq[0:1, cs], ssq_ps[ch][:], 1.0 / D)

    # var = msq - mu^2 ; sd = sqrt(var + eps) ; rstd = 1/sd ; -mu*rstd
    # chunked so early chunks' broadcast matmuls start before late chunks'
    # stats finish (cuts the serial row-chain latency out of the LN span)
    tmp = sb.tile([1, width], F32, tag="ln_tmp", name="ln_tmp")
    amb = sb.tile([1, width], BF16, tag="ln_amb", name="ln_amb")
    bmb = sb.tile([2, width], BF16, tag="ln_bmb", name="ln_bmb")
    for ch in range(ch_n):
        cs = slice(512 * ch, 512 * ch + 512)
        nc.vector.tensor_mul(tmp[0:1, cs], mu[0:1, cs], mu[0:1, cs])
        nc.vector.tensor_sub(msq[0:1, cs], msq[0:1, cs], tmp[0:1, cs])
        nc.vector.tensor_scalar_add(msq[0:1, cs], msq[0:1, cs], EPS)
        nc.scalar.activation(msq[0:1, cs], msq[0:1, cs], Sqrt)
        nc.vector.reciprocal(am[0:1, cs], msq[0:1, cs])
        nc.vector.scalar_tensor_tensor(bm[0:1, cs], mu[0:1, cs], -1.0,
                                       am[0:1, cs], op0=Mult, op1=Mult)
        nc.vector.tensor_copy(amb[0:1, cs], am[0:1, cs])
        nc.vector.tensor_copy(bmb[0:2, cs], bm[0:2, cs])

    # apply chunk-outer (512 cols of all 6 tiles at a time) so consumers of
    # the first output columns start long before the full apply finishes
    with (
        tc.tile_pool(name="ln_ab_ps", bufs=2, space="PSUM") as ab_pool,
        tc.tile_pool(name="ln_ap", bufs=2) as ap_pool,
    ):
        for ch in range(ch_n):
            cs = slice(512 * ch, 512 * ch + 512)
            for dt in DT_ORDER:
                amat = ab_pool.tile([128, 512], F32, tag="ln_amat",
                                    name="ln_amat")
                bmat = ab_pool.tile([128, 512], F32, tag="ln_bmat",
                                    name="ln_bmat")
                nc.tensor.matmul(amat[:],
                                 gb_sb[0:1, 128 * dt:128 * dt + 128],
                                 amb[0:1, cs], start=True, stop=True)
                nc.tensor.matmul(bmat[:],
                                 gb_sb[0:2, 128 * dt:128 * dt + 128],
                                 bmb[0:2, cs], start=True, stop=True)
                if out_f is not None:
                    dst = out_f[dt]
                    nc.vector.tensor_mul(dst[:, cs], r_tiles[dt][:, cs],
                                         amat[:])
                    nc.vector.tensor_add(dst[:, cs], dst[:, cs], bmat[:])
                    if out_b is not None:
                        nc.vector.tensor_copy(out_b[dt][:, cs],
                                              out_f[dt][:, cs])
                else:
                    # f32 intermediate: only one bf16 rounding on the output
                    tmpa = ap_pool.tile([128, 512], F32, tag="ln_apf",
                                        name="ln_apf")
                    nc.vector.tensor_mul(tmpa[:], r_tiles[dt][:, cs],
                                         amat[:])
                    nc.vector.tensor_add(out_b[dt][:, cs], tmpa[:], bmat[:])


def _attention(tc, ctx, x_tiles, wq_sb, wv_sb, mask_tiles, ag_in, ident_sb,
               causal, on_pair=None, dt_order=None):
    """One attention block in transposed layout (all-bf16 matmul operands).

    x_tiles: 6 x (128, S) bf16 SBUF tiles (caller-owned).
    causal=True skips fully-masked q-regions at 128-column granularity and
    applies mask_tiles (16 x (128, 128) bf16 additive diagonal-block mask)
    via identity-matmul accumulation into the score PSUM.
    dt_order: projection contraction order (to match the order the caller's
    x tiles become ready).
    Writes o^T for this core's 6 heads (384, S) bf16 into ag_in DRAM.
    """
    nc = tc.nc
    if dt_order is None:
        dt_order = list(range(DT))

    qkv_pool = ctx.enter_context(tc.tile_pool(name="attn_qkv", bufs=1))
    kv_pool = ctx.enter_context(tc.tile_pool(name="attn_kv", bufs=1))
    qt_sb = [qkv_pool.tile([128, S], BF16, tag=f"qt{p}", name=f"qt{p}")
             for p in range(NPAIR)]
    kvt_sb = [qkv_pool.tile([128, S], BF16, tag=f"kvt{p}", name=f"kvt{p}")
              for p in range(NPAIR)]
    kv_sb = [kv_pool.tile([128, NPAIR * 128], BF16, tag=f"kv{kt}",
                          name=f"kv{kt}") for kt in range(KT)]

    # projections, dt-outer so x tiles are consumed as they become ready
    with (
        tc.tile_pool(name="attn_proj_ps", bufs=1, space="PSUM") as pps,
        tc.tile_pool(name="attn_tr_ps", bufs=2, space="PSUM") as tps,
    ):
        for qc in range(QC):
            qs = slice(512 * qc, 512 * qc + 512)
            tiles = [pps.tile([128, 512], F32, tag=f"proj{j}",
                              name=f"proj{j}") for j in range(2 * NPAIR)]
            for i, dt in enumerate(dt_order):
                j = 0
                for p in range(NPAIR):
                    for wsb in (wq_sb, wv_sb):
                        nc.tensor.matmul(
                            tiles[j][:], wsb[dt][:, 128 * p:128 * p + 128],
                            x_tiles[dt][:, qs],
                            start=(i == 0), stop=(i == DT - 1))
                        j += 1
            j = 0
            for p in range(NPAIR):
                for dst in (qt_sb, kvt_sb):
                    nc.vector.tensor_copy(dst[p][:, qs], tiles[j][:])
                    j += 1
            # kv token-layout tiles via PE transpose of kv^T
            for kt in range(4 * qc, 4 * qc + 4):
                tp = tps.tile([128, NPAIR * 128], BF16, tag="tr", name="tr")
                for p in range(NPAIR):
                    nc.tensor.matmul(
                        tp[:, 128 * p:128 * p + 128],
                        kvt_sb[p][:, 128 * kt:128 * kt + 128],
                        ident_sb[:], is_transpose=True,
                        start=True, stop=True)
                nc.vector.tensor_copy(kv_sb[kt][:], tp[:])

    # attention proper, one head-pair at a time.
    # PSUM: ot (128,2048)f32 = 4 banks; wt (128,1024)f32 x 2 bufs = 4 banks.
    with (
        tc.tile_pool(name="attn_wt_ps", bufs=2, space="PSUM") as wt_pool,
        tc.tile_pool(name="attn_ot_ps", bufs=1, space="PSUM") as ot_pool,
        tc.tile_pool(name="attn_sc", bufs=3) as sc_pool,
        tc.tile_pool(name="attn_rs", bufs=8) as rs_pool,
        tc.tile_pool(name="attn_o", bufs=2) as o_pool,
    ):
        for p in range(NPAIR):
            ot = ot_pool.tile([128, S], F32, tag="ot", name="ot")
            for kt in range(KT):
                ksl = slice(128 * kt, 128 * kt + 128)
                dq = kt // 4 if causal else 0
                r128 = kt % 4 if causal else 0
                win = 128 * r128 + 128
                heads = {}
                for hi, (plo, phi) in enumerate(((0, 64), (64, 128))):
                    score = sc_pool.tile([128, S], BF16, tag=f"sc{hi}",
                                         name=f"sc{hi}")
                    rsh = rs_pool.tile([128, 2], F32, tag=f"rsh{hi}",
                                       name=f"rsh{hi}")
                    nhalf = 0
                    for half in range(2):
                        # live columns start at the 128-block diagonal edge
                        lo = max(512 * dq + 128 * r128, 1024 * half)
                        hhi = 1024 * (half + 1)
                        if lo >= hhi:
                            continue
                        base = 1024 * half
                        wt = wt_pool.tile([128, 1024], F32, tag="wt",
                                          name="wt")
                        for qc2 in range(max(dq, 2 * half), 2 * half + 2):
                            w0 = 512 * qc2 - base
                            q0 = 512 * qc2
                            if causal and qc2 == dq:
                                # diagonal 128-block: additive mask (resets
                                # PSUM), QK accumulates on top; then plain
                                # QK for the fully-live suffix
                                nc.tensor.matmul(
                                    wt[:, w0 + 128 * r128:w0 + win],
                                    ident_sb[:], mask_tiles[kt][:],
                                    start=True, stop=False)
                                nc.tensor.matmul(
                                    wt[:, w0 + 128 * r128:w0 + win],
                                    kvt_sb[p][plo:phi, ksl],
                                    qt_sb[p][plo:phi,
                                             q0 + 128 * r128:q0 + win],
                                    start=False, stop=True,
                                    tile_position=(plo, 0))
                                if win < 512:
                                    nc.tensor.matmul(
                                        wt[:, w0 + win:w0 + 512],
                                        kvt_sb[p][plo:phi, ksl],
                                        qt_sb[p][plo:phi, q0 + win:q0 + 512],
                                        start=True, stop=True,
                                        tile_position=(plo, 0))
                            else:
                                nc.tensor.matmul(
                                    wt[:, w0:w0 + 512],
                                    kvt_sb[p][plo:phi, ksl],
                                    qt_sb[p][plo:phi, q0:q0 + 512],
                                    start=True, stop=True,
                                    tile_position=(plo, 0))
                        nc.scalar.activation(
                            score[:, lo:hhi], wt[:, lo - base:1024],
                            Exp, accum_out=rsh[:, nhalf:nhalf + 1])
                        nhalf += 1
                    if nhalf == 2:
                        rs = rs_pool.tile([128, 1], F32, tag=f"rs{hi}",
                                          name=f"rs{hi}")
                        nc.vector.tensor_add(rs[:], rsh[:, 0:1], rsh[:, 1:2])
                        rs_ap = rs[:]
                    else:
                        rs_ap = rsh[:, 0:1]
                    ri = rs_pool.tile([128, 1], F32, tag=f"ri{hi}",
                                      name=f"ri{hi}")
                    nc.vector.reciprocal(ri[:], rs_ap)
                    kvs = rs_pool.tile([128, DH], BF16, tag=f"kvs{hi}",
                                       name=f"kvs{hi}")
                    h_local = 2 * p + hi
                    nc.vector.tensor_scalar_mul(
                        kvs[:], kv_sb[kt][:, DH * h_local:DH * h_local + DH],
                        ri[:])
                    heads[hi] = (score, kvs, dq)
                for hi, (plo, phi) in enumerate(((0, 64), (64, 128))):
                    score, kvs, dq = heads[hi]
                    for qc2 in range(dq, QC):
                        q0 = 512 * qc2
                        # on the diagonal k-tile, skip the score columns
                        # left of the 128-block edge (zero / never written);
                        # they were started by earlier k-tiles
                        c0 = q0 + 128 * r128 if (causal and qc2 == dq) else q0
                        stop_kt = 4 * qc2 + 3 if causal else KT - 1
                        nc.tensor.matmul(ot[plo:phi, c0:q0 + 512], kvs[:],
                                         score[:, c0:q0 + 512],
                                         start=(kt == 0),
                                         stop=(kt == stop_kt),
                                         skip_group_check=causal,
                                         tile_position=(0, plo))
            o_sb = o_pool.tile([128, S], BF16, tag="o", name="o")
            nc.vector.tensor_copy(o_sb[:], ot[:])
            nc.sync.dma_start(ag_in[128 * p:128 * p + 128, :], o_sb[:])
            if on_pair is not None:
                on_pair(p)


def build(nc, stage="full", reps=1):
    xTb = nc.dram_tensor("xTb", [D, S], BF16, kind="ExternalInput")
    xT = nc.dram_tensor("xT", [D, S], F32, kind="ExternalInput")
    maskc = nc.dram_tensor("maskc", [S, 128], BF16, kind="ExternalInput")
    ident = nc.dram_tensor("ident", [128, 128], BF16, kind="ExternalInput")
    wq1 = nc.dram_tensor("wq1", [D, HLOC * DH], BF16, kind="ExternalInput")
    wv1 = nc.dram_tensor("wv1", [D, HLOC * DH], BF16, kind="ExternalInput")
    wq2 = nc.dram_tensor("wq2", [D, HLOC * DH], BF16, kind="ExternalInput")
    wv2 = nc.dram_tensor("wv2", [D, HLOC * DH], BF16, kind="ExternalInput")
    w1 = nc.dram_tensor("w1", [D, DFF], BF16, kind="ExternalInput")
    w2 = nc.dram_tensor("w2", [DFF, D], BF16, kind="ExternalInput")
    b1c = nc.dram_tensor("b1c", [DFF, 1], F32, kind="ExternalInput")
    b2c = nc.dram_tensor("b2c", [D, 1], F32, kind="ExternalInput")
    gb1 = nc.dram_tensor("gb1", [2, D], BF16, kind="ExternalInput")
    gb2 = nc.dram_tensor("gb2", [2, D], BF16, kind="ExternalInput")
    gbf = nc.dram_tensor("gbf", [2, D], BF16, kind="ExternalInput")
    cb = nc.dram_tensor("cb", [1, 1], I32, kind="ExternalInput")
    rb = nc.dram_tensor("rb", [1, 1], I32, kind="ExternalInput")

    ag1_in = nc.dram_tensor("ag1_in", [NPAIR * 128, S], BF16)
    ag1_outs = [nc.dram_tensor(f"ag1_out{p}", [NCORES * 128, S], BF16,
                               addr_space="Shared") for p in range(NPAIR)]
    x2s = nc.dram_tensor("x2s", [D, S], BF16)
    ag2_in = nc.dram_tensor("ag2_in", [NPAIR * 128, S], BF16)
    ag2_outs = [nc.dram_tensor(f"ag2_out{p}", [NCORES * 128, S], BF16,
                               addr_space="Shared") for p in range(NPAIR)]

    if stage in ("x2", "b1", "b1nm"):
        dbg = nc.dram_tensor("dbg", [D, S], F32, kind="ExternalOutput")
    elif stage == "x3":
        dbg = nc.dram_tensor("dbg", [D, SH], F32, kind="ExternalOutput")
    outT = None
    if stage in ("full", "sim", "fullnc"):
        outT = nc.dram_tensor("outT", [D, SH], F32, kind="ExternalOutput")

    rg = RG

    with tile.TileContext(nc) as tc:
        cv = _scalar_from_input(nc, cb, SH)
        rv = _scalar_from_input(nc, rb, 256 * (B - 1))
        for _rep in range(reps):
            _build_body(tc, nc, stage, cv, rv, locals())


def _all_gather_pair(nc, stage, rg, ag_in, ag_out_p, p):
    """AllGather one head-pair's slice within the 2-core batch group
    (emitted as soon as pair p's o^T is in DRAM, so earlier pairs'
    exchange overlaps later pairs' compute)."""
    in_ap = ag_in[128 * p:128 * p + 128, :]
    if stage.startswith("sim") or stage.startswith("fullnc"):
        nc.sync.dma_start(ag_out_p[0:128, :], in_ap)
        nc.sync.dma_start(ag_out_p[128:256, :], in_ap)
    else:
        nc.gpsimd.collective_compute(
            "AllGather", mybir.AluOpType.bypass, replica_groups=rg,
            ins=[in_ap.opt()], outs=[ag_out_p[:].opt()])


def _build_body(tc, nc, stage, cv, rv, env):
    (xTb, xT, maskc, wq1, wv1, wq2, wv2, w1, w2, b1c, b2c, gb1, gb2,
     gbf, x2s, ag1_in, ag1_outs, ag2_in, ag2_outs, rg, ident) = (
        env["xTb"], env["xT"], env["maskc"], env["wq1"], env["wv1"],
        env["wq2"], env["wv2"], env["w1"], env["w2"], env["b1c"],
        env["b2c"], env["gb1"], env["gb2"], env["gbf"], env["x2s"],
        env["ag1_in"], env["ag1_outs"], env["ag2_in"], env["ag2_outs"],
        env["rg"], env["ident"])
    dbg = env.get("dbg")
    outT = env.get("outT")
    with ExitStack() as top:
        const_pool = top.enter_context(tc.tile_pool(name="const", bufs=1))
        ones_b = const_pool.tile([128, 1], BF16, tag="ones_b", name="ones_b")
        nc.vector.memset(ones_b[:], 1.0)
        gb_sb = {}
        for nm, dram in (("gb1", gb1), ("gb2", gb2), ("gbf", gbf)):
            t = const_pool.tile([2, D], BF16, tag=nm, name=nm)
            nc.scalar.dma_start(t[:], dram[:])
            gb_sb[nm] = t
        ident_sb = const_pool.tile([128, 128], BF16, tag="ident", name="ident")
        nc.scalar.dma_start(ident_sb[:], ident[:])

        # ---------------- block 1 ----------------
        # pools that outlive the block-1 scope (stack-ordered before it)
        wpool2 = top.enter_context(tc.tile_pool(name="w2p", bufs=1))
        x2b_pool = top.enter_context(tc.tile_pool(name="x2b", bufs=1))
        with ExitStack() as blk1_outer:
            xb_pool = blk1_outer.enter_context(
                tc.tile_pool(name="xb", bufs=1))
            xb = [xb_pool.tile([128, S], BF16, tag=f"x{dt}", name=f"x{dt}")
                  for dt in range(DT)]
            with ExitStack() as blk1:
                # load order: weights (small, needed first by the dt-outer
                # projection), then x, then mask (needed ~35us later)
                wpool = blk1.enter_context(tc.tile_pool(name="w1p", bufs=1))
                wq_sb, wv_sb = [], []
                for dt in range(DT):
                    wq = wpool.tile([128, HLOC * DH], BF16, tag=f"wq{dt}",
                                    name=f"wq{dt}")
                    nc.sync.dma_start(wq[:], wq1[128 * dt:128 * dt + 128, :])
                    wq_sb.append(wq)
                    wv = wpool.tile([128, HLOC * DH], BF16, tag=f"wv{dt}",
                                    name=f"wv{dt}")
                    nc.sync.dma_start(wv[:], wv1[128 * dt:128 * dt + 128, :])
                    wv_sb.append(wv)
                    nc.sync.dma_start(xb[dt][:],
                                      xTb[128 * dt:128 * dt + 128, :])
                xf = [xb_pool.tile([128, S], F32, tag=f"xf{dt}",
                                   name=f"xf{dt}") for dt in range(DT)]
                for dt in range(DT):
                    nc.sync.dma_start(xf[dt][:],
                                      xT[128 * dt:128 * dt + 128, :])
                m_tiles = None
                if stage != "b1nm":
                    mask_pool = blk1.enter_context(
                        tc.tile_pool(name="mask", bufs=1))
                    m_tiles = []
                    for kt in range(KT):
                        m = mask_pool.tile([128, 128], BF16, tag=f"m{kt}",
                                           name=f"m{kt}")
                        nc.sync.dma_start(
                            m[:], maskc[128 * kt:128 * kt + 128, :])
                        m_tiles.append(m)
                _attention(tc, blk1, xb, wq_sb, wv_sb, m_tiles, ag1_in,
                           ident_sb, causal=(stage != "b1nm"),
                           on_pair=lambda p: _all_gather_pair(
                               nc, stage, rg, ag1_in, ag1_outs[p], p))

            if stage in ("b1", "b1nm"):
                with tc.tile_pool(name="b1dbg", bufs=2) as dp:
                    for dt in range(DT):
                        t = dp.tile([128, S], BF16, tag="d", name="d")
                        nc.sync.dma_start(
                            t[:], ag1_outs[dt % NPAIR][
                                bass.ds(rv + 128 * (dt // NPAIR), 128), :])
                        tf = dp.tile([128, S], F32, tag="df", name="df")
                        nc.vector.tensor_copy(tf[:], t[:])
                        nc.sync.dma_start(dbg[128 * dt:128 * dt + 128, :],
                                          tf[:])
                return

            # ---------------- LN1 -> x2 ----------------
            # prefetch block-2 weights during the gather window
            w2q_sb, w2v_sb = [], []
            for dt in range(DT):
                wq = wpool2.tile([128, HLOC * DH], BF16, tag=f"wq{dt}",
                                 name=f"wq{dt}")
                nc.scalar.dma_start(wq[:], wq2[128 * dt:128 * dt + 128, :])
                w2q_sb.append(wq)
            for dt in range(DT):
                wv = wpool2.tile([128, HLOC * DH], BF16, tag=f"wv{dt}",
                                 name=f"wv{dt}")
                nc.scalar.dma_start(wv[:], wv2[128 * dt:128 * dt + 128, :])
                w2v_sb.append(wv)

            x2b = [x2b_pool.tile([128, S], BF16, tag=f"x2b{dt}",
                                 name=f"x2b{dt}") for dt in range(DT)]
            with ExitStack() as lctx:
                rp = lctx.enter_context(tc.tile_pool(name="ln1_r", bufs=1))
                tp = lctx.enter_context(tc.tile_pool(name="ln1_t", bufs=2))
                r_tiles = [None] * DT
                for i, dt in enumerate(DT_ORDER):
                    t1 = tp.tile([128, S], BF16, tag="ag", name="ag")
                    eng = nc.sync if i % 2 == 0 else nc.scalar
                    eng.dma_start(
                        t1[:], ag1_outs[dt % NPAIR][
                            bass.ds(rv + 128 * (dt // NPAIR), 128), :])
                    r = rp.tile([128, S], BF16, tag=f"r{dt}", name=f"r{dt}")
                    nc.vector.tensor_add(r[:], t1[:], xf[dt][:])
                    r_tiles[dt] = r
                _layernorm(tc, lctx, r_tiles, gb_sb["gb1"], ones_b, S,
                           None, x2b)

        # spill x2 for the LN2 residual read-back (dynamic column half)
        for dt in range(DT):
            nc.sync.dma_start(x2s[128 * dt:128 * dt + 128, :], x2b[dt][:])

        if stage == "x2":
            with tc.tile_pool(name="x2dbg", bufs=2) as dp:
                for dt in range(DT):
                    tf = dp.tile([128, S], F32, tag="df", name="df")
                    nc.vector.tensor_copy(tf[:], x2b[dt][:])
                    nc.sync.dma_start(dbg[128 * dt:128 * dt + 128, :], tf[:])
            return

        # ---------------- block 2 ----------------
        # prefetch FFN w1 + biases on the Act queue (idle during proj)
        b1_sb, b2_sb, w1_sb = [], [], []
        if stage in ("full", "sim", "fullnc"):
            b_pool = top.enter_context(tc.tile_pool(name="ffn_b", bufs=1))
            w1_pool = top.enter_context(tc.tile_pool(name="ffn_w1", bufs=1))
            for ft in range(FT):
                bt = b_pool.tile([128, 1], F32, tag=f"b1_{ft}",
                                 name=f"b1_{ft}")
                nc.scalar.dma_start(bt[:], b1c[128 * ft:128 * ft + 128, :])
                b1_sb.append(bt)
            for dt in range(DT):
                bt = b_pool.tile([128, 1], F32, tag=f"b2_{dt}",
                                 name=f"b2_{dt}")
                nc.scalar.dma_start(bt[:], b2c[128 * dt:128 * dt + 128, :])
                b2_sb.append(bt)
            for dt in range(DT):
                wt = w1_pool.tile([128, DFF], BF16, tag=f"w1_{dt}",
                                  name=f"w1_{dt}")
                nc.scalar.dma_start(wt[:], w1[128 * dt:128 * dt + 128, :])
                w1_sb.append(wt)

        with ExitStack() as blk2:
            _attention(tc, blk2, x2b, w2q_sb, w2v_sb, None, ag2_in,
                       ident_sb, causal=False,
                       on_pair=lambda p: _all_gather_pair(
                           nc, stage, rg, ag2_in, ag2_outs[p], p),
                       dt_order=DT_ORDER)

        # ---------------- LN2 -> x3 (token half) ----------------
        x3_pool = top.enter_context(tc.tile_pool(name="x3", bufs=1))
        x3b = [x3_pool.tile([128, SH], BF16, tag=f"x3b{dt}", name=f"x3b{dt}")
               for dt in range(DT)]
        with ExitStack() as lctx:
            rp = lctx.enter_context(tc.tile_pool(name="ln2_r", bufs=1))
            tp = lctx.enter_context(tc.tile_pool(name="ln2_t", bufs=2))
            r_tiles = [None] * DT
            for i, dt in enumerate(DT_ORDER):
                t1 = tp.tile([128, SH], BF16, tag="ag", name="ag")
                t2 = tp.tile([128, SH], BF16, tag="xres", name="xres")
                nc.sync.dma_start(
                    t1[:], ag2_outs[dt % NPAIR][
                        bass.ds(rv + 128 * (dt // NPAIR), 128),
                        bass.ds(cv, SH)])
                nc.scalar.dma_start(
                    t2[:], x2s[128 * dt:128 * dt + 128, bass.ds(cv, SH)])
                r = rp.tile([128, SH], BF16, tag=f"r{dt}", name=f"r{dt}")
                nc.vector.tensor_add(r[:], t1[:], t2[:])
                r_tiles[dt] = r
            _layernorm(tc, lctx, r_tiles, gb_sb["gb2"], ones_b, SH,
                       None, x3b)

        if stage == "x3":
            with tc.tile_pool(name="x3dbg", bufs=2) as dp:
                for dt in range(DT):
                    tf = dp.tile([128, SH], F32, tag="df", name="df")
                    nc.vector.tensor_copy(tf[:], x3b[dt][:])
                    nc.sync.dma_start(dbg[128 * dt:128 * dt + 128, :], tf[:])
            return

        # ---------------- FFN ----------------
        r3_pool = top.enter_context(tc.tile_pool(name="r3", bufs=1))
        r3 = [r3_pool.tile([128, SH], F32, tag=f"r3{dt}", name=f"r3{dt}")
              for dt in range(DT)]
        with ExitStack() as ffn_stack:
            w2_pool = ffn_stack.enter_context(
                tc.tile_pool(name="ffn_w2", bufs=1))
            w2_sb = []
            for ft in range(FT):
                wt = w2_pool.tile([128, D], BF16, tag=f"w2_{ft}",
                                  name=f"w2_{ft}")
                nc.scalar.dma_start(wt[:], w2[128 * ft:128 * ft + 128, :])
                w2_sb.append(wt)
            h_pool = ffn_stack.enter_context(
                tc.tile_pool(name="ffn_h", bufs=3))
            with (
                tc.tile_pool(name="ffn_h_ps", bufs=2, space="PSUM") as hps,
                tc.tile_pool(name="ffn_y_ps", bufs=1, space="PSUM") as yps,
            ):
                for ch in range(SH // 512):
                    cs = slice(512 * ch, 512 * ch + 512)
                    y_ps = [yps.tile([128, 512], F32, tag=f"yp{dt}",
                                     name=f"yp{dt}") for dt in range(DT)]
                    for ft in range(FT):
                        ps = hps.tile([128, 512], F32, tag="hp", name="hp")
                        for i, dt in enumerate(DT_ORDER):
                            nc.tensor.matmul(
                                ps[:], w1_sb[dt][:, 128 * ft:128 * ft + 128],
                                x3b[dt][:, cs],
                                start=(i == 0), stop=(i == DT - 1))
                        h = h_pool.tile([128, 512], BF16, tag="h", name="h")
                        nc.vector.tensor_scalar(h[:], ps[:], b1_sb[ft][:],
                                                0.0, op0=Add, op1=Max)
                        for dt in range(DT):
                            nc.tensor.matmul(
                                y_ps[dt][:],
                                w2_sb[ft][:, 128 * dt:128 * dt + 128],
                                h[:],
                                start=(ft == 0), stop=(ft == FT - 1))
                    for dt in range(DT):
                        nc.vector.scalar_tensor_tensor(
                            r3[dt][:, cs], y_ps[dt][:], b2_sb[dt][:],
                            x3b[dt][:, cs], op0=Add, op1=Add)

        # ---------------- LN3 -> out ----------------
        with ExitStack() as lctx:
            ofin = [r3_pool.tile([128, SH], F32, tag=f"of{dt}",
                                 name=f"of{dt}") for dt in range(DT)]
            _layernorm(tc, lctx, r3, gb_sb["gbf"], ones_b, SH, ofin, None)
            for dt in range(DT):
                for ch in range(SH // 512):
                    cs = slice(512 * ch, 512 * ch + 512)
                    nc.sync.dma_start(outT[128 * dt:128 * dt + 128, cs],
                                      ofin[dt][:, cs])


_CACHE = {}


def _get_compiled(stage="full"):
    if stage not in _CACHE:
        reps = 1
        name = stage
        import re as _re
        m = _re.match(r"^(.*)_r(\d+)$", stage)
        if m:
            name, reps = m.group(1), int(m.group(2))
        ndev = 1 if name.startswith("sim") else NCORES
        nc = bacc.Bacc("TRN2", target_bir_lowering=False, debug=False,
                       num_devices=ndev)
        build(nc, name, reps=reps)
        nc.compile()
        _CACHE[stage] = nc
    return _CACHE[stage]


def make_in_maps(x, mask, Wq1, Wv1, g1, be1, Wq2, Wv2, g2, be2,
                 Wf1, bf1, Wf2, bf2, gf, bef):
    x = np.asarray(x, np.float32)
    mask = np.asarray(mask)
    maskT = np.where(np.asarray(mask[0, 0]).T, np.float32(-1e9),
                     np.float32(0.0))
    # per-k-tile diagonal 128-block of the additive mask
    maskc = np.empty((S, 128), np.float32)
    for kt in range(KT):
        c0 = 128 * kt
        maskc[128 * kt:128 * kt + 128] = maskT[128 * kt:128 * kt + 128,
                                               c0:c0 + 128]
    maskc = maskc.astype(NP_BF16)
    w1b = np.asarray(Wf1, np.float32).astype(NP_BF16)
    w2b = np.asarray(Wf2, np.float32).astype(NP_BF16)
    scale = np.float32(1.0 / SQRT_DK)
    in_maps = []
    for c in range(NCORES):
        b, hh = c // 2, c % 2
        cols = slice(HLOC * DH * hh, HLOC * DH * (hh + 1))
        xTf = np.ascontiguousarray(x[b].T)
        in_maps.append({
            "xTb": xTf.astype(NP_BF16),
            "xT": xTf,
            "ident": np.eye(128, dtype=np.float32).astype(NP_BF16),
            "maskc": maskc,
            # fold the 1/sqrt(dk) into the Q projection
            "wq1": (np.ascontiguousarray(
                np.asarray(Wq1, np.float32)[:, cols]) * scale).astype(NP_BF16),
            "wv1": np.ascontiguousarray(
                np.asarray(Wv1, np.float32)[:, cols]).astype(NP_BF16),
            "wq2": (np.ascontiguousarray(
                np.asarray(Wq2, np.float32)[:, cols]) * scale).astype(NP_BF16),
            "wv2": np.ascontiguousarray(
                np.asarray(Wv2, np.float32)[:, cols]).astype(NP_BF16),
            "w1": w1b,
            "w2": w2b,
            "b1c": np.asarray(bf1, np.float32).reshape(DFF, 1),
            "b2c": np.asarray(bf2, np.float32).reshape(D, 1),
            "gb1": np.stack([np.asarray(g1, np.float32),
                             np.asarray(be1, np.float32)]).astype(NP_BF16),
            "gb2": np.stack([np.asarray(g2, np.float32),
                             np.asarray(be2, np.float32)]).astype(NP_BF16),
            "gbf": np.stack([np.asarray(gf, np.float32),
                             np.asarray(bef, np.float32)]).astype(NP_BF16),
            "cb": np.array([[SH * hh]], np.int32),
            "rb": np.array([[256 * b]], np.int32),
        })
    return in_maps


def run_spmd(in_maps, stage="full"):
    nc = _get_compiled(stage)
    return bass_utils.run_bass_kernel_spmd(nc, in_maps,
                                           core_ids=list(range(NCORES)))


def kernel(**inputs):
    in_maps = make_in_maps(**inputs)
    res = run_spmd(in_maps, "full")
    out = np.empty((B, S, D), np.float32)
    for c in range(NCORES):
        b, hh = c // 2, c % 2
        out[b, SH * hh:SH * (hh + 1), :] = res.results[c]["outT"].T
    return out


class _Runner:
    """Reusable jitted dispatcher (mirrors bass2jax.run_bass_via_pjrt's
    multi-core path) so repeated executions skip re-tracing and host
    transfers — used for timing."""

    def __init__(self, stage="full"):
        import jax
        from jax.sharding import Mesh, PartitionSpec
        from jax.experimental.shard_map import shard_map
        from concourse import bass2jax as b2j

        b2j.install_neuronx_cc_hook()
        nc = _get_compiled(stage)
        pname = (nc.partition_id_tensor.name
                 if nc.partition_id_tensor else None)
        in_names, out_names, out_avals = [], [], []
        for alloc in nc.m.functions[0].allocations:
            if not isinstance(alloc, mybir.MemoryLocationSet):
                continue
            name = alloc.memorylocations[0].name
            if alloc.kind == "ExternalInput":
                if name != pname:
                    in_names.append(name)
            elif alloc.kind == "ExternalOutput":
                out_names.append(name)
                out_avals.append(jax.core.ShapedArray(
                    tuple(alloc.tensor_shape), mybir.dt.np(alloc.dtype)))
        self.in_names, self.out_names = list(in_names), list(out_names)
        self.out_avals = out_avals
        all_in = in_names + out_names
        if pname is not None:
            all_in = all_in + [pname]
        n_params, n_outs = len(in_names), len(out_names)

        def _body(*args):
            operands = list(args)
            if pname is not None:
                operands.append(b2j.partition_id_tensor())
            outs = b2j._bass_exec_p.bind(
                *operands, out_avals=tuple(out_avals), in_names=tuple(all_in),
                out_names=tuple(out_names), lowering_input_output_aliases=(),
                sim_require_finite=True, sim_require_nnan=True, nc=nc)
            return tuple(outs)

        devices = jax.devices()[:NCORES]
        mesh = Mesh(np.asarray(devices), ("core",))
        in_specs = (PartitionSpec("core"),) * (n_params + n_outs)
        out_specs = (PartitionSpec("core"),) * n_outs
        self.fn = jax.jit(
            shard_map(_body, mesh=mesh, in_specs=in_specs,
                      out_specs=out_specs, check_rep=False),
            donate_argnums=tuple(range(n_params, n_params + n_outs)),
            keep_unused=True)
        self._jax = jax

    def device_inputs(self, in_maps):
        import jax
        concat = [np.concatenate([np.asarray(in_maps[c][n])
                                  for c in range(NCORES)], axis=0)
                  for n in self.in_names]
        return [jax.device_put(a) for a in concat]

    def zero_outs(self):
        import jax.numpy as jnp
        return [jnp.zeros((NCORES * av.shape[0], *av.shape[1:]), av.dtype)
                for av in self.out_avals]

    def __call__(self, dev_in, zeros):
        return self.fn(*dev_in, *zeros)


class _RunnerNZ:
    """Timing runner: zero output buffers are created inside the shard_map
    body (device-local), so repeated calls move no host data at all."""

    def __init__(self, stage="full"):
        import jax
        import jax.numpy as jnp
        from jax.sharding import Mesh, PartitionSpec
        from jax.experimental.shard_map import shard_map
        from concourse import bass2jax as b2j

        b2j.install_neuronx_cc_hook()
        nc = _get_compiled(stage)
        pname = (nc.partition_id_tensor.name
                 if nc.partition_id_tensor else None)
        in_names, out_names, out_avals = [], [], []
        for alloc in nc.m.functions[0].allocations:
            if not isinstance(alloc, mybir.MemoryLocationSet):
                continue
            name = alloc.memorylocations[0].name
            if alloc.kind == "ExternalInput":
                if name != pname:
                    in_names.append(name)
            elif alloc.kind == "ExternalOutput":
                out_names.append(name)
                out_avals.append(jax.core.ShapedArray(
                    tuple(alloc.tensor_shape), mybir.dt.np(alloc.dtype)))
        self.in_names, self.out_names = in_names, out_names
        all_in = in_names + out_names
        if pname is not None:
            all_in = all_in + [pname]

        def _body(*args):
            operands = list(args)
            operands += [jnp.zeros(av.shape, av.dtype) for av in out_avals]
            if pname is not None:
                operands.append(b2j.partition_id_tensor())
            outs = b2j._bass_exec_p.bind(
                *operands, out_avals=tuple(out_avals), in_names=tuple(all_in),
                out_names=tuple(out_names), lowering_input_output_aliases=(),
                sim_require_finite=True, sim_require_nnan=True, nc=nc)
            return tuple(outs)

        devices = jax.devices()[:NCORES]
        mesh = Mesh(np.asarray(devices), ("core",))
        self.fn = jax.jit(
            shard_map(_body, mesh=mesh,
                      in_specs=(PartitionSpec("core"),) * len(in_names),
                      out_specs=(PartitionSpec("core"),) * len(out_names),
                      check_rep=False),
            keep_unused=True)

    def device_inputs(self, in_maps):
        import jax
        concat = [np.concatenate([np.asarray(in_maps[c][n])
                                  for c in range(NCORES)], axis=0)
                  for n in self.in_names]
        return [jax.device_put(a) for a in concat]

    def __call__(self, dev_in):
        return self.fn(*dev_in)


# revision 34
# speedup vs baseline: 2.8519x; 1.0158x over previous
"""Trainium2 Bass kernel for a 2-attention-block + FFN decoder stack.

Shapes: x (4, 2048, 768), 12 heads x 64, d_ff 3072.
Sharding over 8 cores: core c handles batch b=c//2 and heads 6*(c%2)..+6 for
both attention blocks; the final FFN+LN runs on token half c%2 of batch b.
Per-pair bf16 AllGathers (replica groups [[0,1],[2,3],...]) exchange the
per-head attention outputs so each core can LayerNorm over the full model
dim.

All compute is done in "transposed" layout (D on partitions, tokens on the
free axis).  The source model's softmax runs over the *query* axis (dim=2
quirk), which in transposed layout (k on partitions, q on free axis) is a
per-partition-row softmax: exp on ScalarE with accum_out produces the row
sums for free; the 1/rowsum is folded into the (tiny) KV matrix instead of
the (huge) score matrix.  No max-subtraction is needed: |w| stays O(10) so
exp cannot overflow, and softmax is shift-invariant.

Block-1 exploits the causal mask: fully-masked 512-wide q-chunks of each
128-row k-tile are skipped entirely (no QK, no exp, no AV -- exp(-1e9)=0
contributes nothing to row sums or AV), and the additive mask matmul runs
only on the diagonal chunk.  kv in token layout comes from PE transposes of
kv^T instead of a second x@Wv matmul.  Projections accumulate with dt as
the outer loop so the preceding LayerNorm's per-tile outputs pipeline
straight into the next block's matmuls.
"""

import os
import sys

for _p in ("/opt/trn_rl_repo", "/root/.axon_site/_ro/trn_rl_repo"):
    if os.path.isdir(_p) and _p not in sys.path:
        sys.path.insert(0, _p)

import numpy as np
from contextlib import ExitStack

from concourse import bass, bacc, mybir, tile
from concourse import bass_utils

F32 = mybir.dt.float32
BF16 = mybir.dt.bfloat16
F8 = mybir.dt.float8e4
I32 = mybir.dt.int32
NP_BF16 = mybir.dt.np(BF16)

B, S, D, H, DH, DFF = 4, 2048, 768, 12, 64, 3072
NCORES = 8
HLOC = 6           # heads per core
NPAIR = 3          # head pairs per core
SQRT_DK = float(np.sqrt(DH))
EPS = 1e-5
SH = S // 2        # token half for FFN
DT = D // 128      # 6 d-tiles
KT = S // 128      # 16 k-tiles
QC = S // 512      # 4 q-chunks
FT = DFF // 128    # 24 ff-tiles

# full 8-rank replica group: 2-core-group collectives are forced onto the
# slow non-Shared path (shared output needs >4 cores), so an 8-rank Shared
# AllGather is faster despite moving 4x the bytes
RG = [list(range(NCORES))]
# LN input processing order: pairs 0,0,1,1,2,2 so the earliest-gathered
# pair's tiles are consumed first
DT_ORDER = [0, 3, 1, 4, 2, 5]

Exp = mybir.ActivationFunctionType.Exp
Sqrt = mybir.ActivationFunctionType.Sqrt
Add = mybir.AluOpType.add
Mult = mybir.AluOpType.mult
Max = mybir.AluOpType.max


def _scalar_from_input(nc, dram, max_val):
    tmp = nc.alloc_registers(f"sv_{dram.name}", mybir.ALL_ENGINES)
    nc.regs_load(tmp, dram[0:1, 0:1])
    return nc.snap(tmp, donate=True, min_val=0, max_val=max_val)


def _layernorm(tc, ctx, r_tiles, gb_sb, ones_b, width, out_f, out_b):
    """LayerNorm over the partition (D) axis of 6 x (128, width) tiles.

    r_tiles may be f32 or bf16; out_f (f32) and out_b (bf16) are optional
    lists of destination tiles.  gb_sb is a (2, 768) bf16 SBUF tile (row 0
    gamma, row 1 beta), applied via tiny outer-product matmuls building
    per-element affine maps.
    """
    nc = tc.nc
    ch_n = width // 512
    sb = ctx.enter_context(tc.tile_pool(name="ln_sb", bufs=1))
    # all row-vector scratch lives at base partition 0 (engine requirement)
    mu = sb.tile([1, width], F32, tag="ln_mu", name="ln_mu")
    msq = sb.tile([1, width], F32, tag="ln_msq", name="ln_msq")
    am = sb.tile([1, width], F32, tag="ln_am", name="ln_am")
    bm = sb.tile([2, width], F32, tag="ln_bm", name="ln_bm")
    nc.vector.memset(bm[0:2, :], 1.0)  # row1 stays ones; row0 overwritten

    with ExitStack() as sctx:
        sq_pool = sctx.enter_context(tc.tile_pool(name="ln_sq", bufs=2))
        ps_pool = sctx.enter_context(
            tc.tile_pool(name="ln_stats_ps", bufs=1, space="PSUM"))
        sum_ps = [ps_pool.tile([1, 512], F32, tag=f"sum{ch}", name=f"sum{ch}")
                  for ch in range(ch_n)]
        ssq_ps = [ps_pool.tile([1, 512], F32, tag=f"ssq{ch}", name=f"ssq{ch}")
                  for ch in range(ch_n)]
        for i, dt in enumerate(DT_ORDER):
            r = r_tiles[dt]
            if r.dtype == BF16:
                rb = r
            else:
                rb = sq_pool.tile([128, width], BF16, tag="rb", name="rb")
                nc.vector.tensor_copy(rb[:], r[:])
            sq = sq_pool.tile([128, width], BF16, tag="sq", name="sq")
            nc.vector.tensor_mul(sq[:], rb[:], rb[:])
            for ch in range(ch_n):
                cs = slice(512 * ch, 512 * ch + 512)
                nc.tensor.matmul(sum_ps[ch][:], ones_b[:, 0:1],
                                 rb[:, cs],
                                 start=(i == 0), stop=(i == DT - 1))
                nc.tensor.matmul(ssq_ps[ch][:], ones_b[:, 0:1],
                                 sq[:, cs],
                                 start=(i == 0), stop=(i == DT - 1))
        for ch in range(ch_n):
            cs = slice(512 * ch, 512 * ch + 512)
            nc.vector.tensor_scalar_mul(mu[0:1, cs], sum_ps[ch][:], 1.0 / D)
            nc.vector.tensor_scalar_mul(msq[0:1, cs], ssq_ps[ch][:], 1.0 / D)

    # var = msq - mu^2 ; sd = sqrt(var + eps) ; rstd = 1/sd ; -mu*rstd
    # chunked so early chunks' broadcast matmuls start before late chunks'
    # stats finish (cuts the serial row-chain latency out of the LN span)
    tmp = sb.tile([1, width], F32, tag="ln_tmp", name="ln_tmp")
    amb = sb.tile([1, width], BF16, tag="ln_amb", name="ln_amb")
    bmb = sb.tile([2, width], BF16, tag="ln_bmb", name="ln_bmb")
    for ch in range(ch_n):
        cs = slice(512 * ch, 512 * ch + 512)
        nc.vector.tensor_mul(tmp[0:1, cs], mu[0:1, cs], mu[0:1, cs])
        nc.vector.tensor_sub(msq[0:1, cs], msq[0:1, cs], tmp[0:1, cs])
        nc.vector.tensor_scalar_add(msq[0:1, cs], msq[0:1, cs], EPS)
        nc.scalar.activation(msq[0:1, cs], msq[0:1, cs], Sqrt)
        nc.vector.reciprocal(am[0:1, cs], msq[0:1, cs])
        nc.vector.scalar_tensor_tensor(bm[0:1, cs], mu[0:1, cs], -1.0,
                                       am[0:1, cs], op0=Mult, op1=Mult)
        nc.vector.tensor_copy(amb[0:1, cs], am[0:1, cs])
        nc.vector.tensor_copy(bmb[0:2, cs], bm[0:2, cs])

    # apply chunk-outer (512 cols of all 6 tiles at a time) so consumers of
    # the first output columns start long before the full apply finishes
    with (
        tc.tile_pool(name="ln_ab_ps", bufs=2, space="PSUM") as ab_pool,
        tc.tile_pool(name="ln_ap", bufs=2) as ap_pool,
    ):
        for ch in range(ch_n):
            cs = slice(512 * ch, 512 * ch + 512)
            for dt in DT_ORDER:
                amat = ab_pool.tile([128, 512], F32, tag="ln_amat",
                                    name="ln_amat")
                bmat = ab_pool.tile([128, 512], F32, tag="ln_bmat",
                                    name="ln_bmat")
                nc.tensor.matmul(amat[:],
                                 gb_sb[0:1, 128 * dt:128 * dt + 128],
                                 amb[0:1, cs], start=True, stop=True)
                nc.tensor.matmul(bmat[:],
                                 gb_sb[0:2, 128 * dt:128 * dt + 128],
                                 bmb[0:2, cs], start=True, stop=True)
                if out_f is not None:
                    dst = out_f[dt]
                    nc.vector.tensor_mul(dst[:, cs], r_tiles[dt][:, cs],
                                         amat[:])
                    nc.vector.tensor_add(dst[:, cs], dst[:, cs], bmat[:])
                    if out_b is not None:
                        nc.vector.tensor_copy(out_b[dt][:, cs],
                                              out_f[dt][:, cs])
                else:
                    # f32 intermediate: only one bf16 rounding on the output
                    tmpa = ap_pool.tile([128, 512], F32, tag="ln_apf",
                                        name="ln_apf")
                    nc.vector.tensor_mul(tmpa[:], r_tiles[dt][:, cs],
                                         amat[:])
                    nc.vector.tensor_add(out_b[dt][:, cs], tmpa[:], bmat[:])


def _attention(tc, ctx, x_tiles, wq_sb, wv_sb, mask_tiles, ag_in, ident_sb,
               causal, on_pair=None, dt_order=None):
    """One attention block in transposed layout (all-bf16 matmul operands).

    x_tiles: 6 x (128, S) bf16 SBUF tiles (caller-owned).
    causal=True skips fully-masked q-regions at 128-column granularity and
    applies mask_tiles (16 x (128, 128) bf16 additive diagonal-block mask)
    via identity-matmul accumulation into the score PSUM.
    dt_order: projection contraction order (to match the order the caller's
    x tiles become ready).
    Writes o^T for this core's 6 heads (384, S) bf16 into ag_in DRAM.
    """
    nc = tc.nc
    if dt_order is None:
        dt_order = list(range(DT))

    qkv_pool = ctx.enter_context(tc.tile_pool(name="attn_qkv", bufs=1))
    kv_pool = ctx.enter_context(tc.tile_pool(name="attn_kv", bufs=1))
    qt_sb = [qkv_pool.tile([128, S], BF16, tag=f"qt{p}", name=f"qt{p}")
             for p in range(NPAIR)]
    kvt_sb = [qkv_pool.tile([128, S], BF16, tag=f"kvt{p}", name=f"kvt{p}")
              for p in range(NPAIR)]
    kv_sb = [kv_pool.tile([128, NPAIR * 128], BF16, tag=f"kv{kt}",
                          name=f"kv{kt}") for kt in range(KT)]

    # projections, dt-outer so x tiles are consumed as they become ready
    with (
        tc.tile_pool(name="attn_proj_ps", bufs=1, space="PSUM") as pps,
        tc.tile_pool(name="attn_tr_ps", bufs=2, space="PSUM") as tps,
    ):
        for qc in range(QC):
            qs = slice(512 * qc, 512 * qc + 512)
            tiles = [pps.tile([128, 512], F32, tag=f"proj{j}",
                              name=f"proj{j}") for j in range(2 * NPAIR)]
            for i, dt in enumerate(dt_order):
                j = 0
                for p in range(NPAIR):
                    for wsb in (wq_sb, wv_sb):
                        nc.tensor.matmul(
                            tiles[j][:], wsb[dt][:, 128 * p:128 * p + 128],
                            x_tiles[dt][:, qs],
                            start=(i == 0), stop=(i == DT - 1))
                        j += 1
            j = 0
            for p in range(NPAIR):
                for dst in (qt_sb, kvt_sb):
                    nc.vector.tensor_copy(dst[p][:, qs], tiles[j][:])
                    j += 1
            # kv token-layout tiles via PE transpose of kv^T
            for kt in range(4 * qc, 4 * qc + 4):
                tp = tps.tile([128, NPAIR * 128], BF16, tag="tr", name="tr")
                for p in range(NPAIR):
                    nc.tensor.matmul(
                        tp[:, 128 * p:128 * p + 128],
                        kvt_sb[p][:, 128 * kt:128 * kt + 128],
                        ident_sb[:], is_transpose=True,
                        start=True, stop=True)
                nc.vector.tensor_copy(kv_sb[kt][:], tp[:])

    # attention proper, one head-pair at a time.
    # PSUM: ot (128,2048)f32 = 4 banks; wt (128,1024)f32 x 2 bufs = 4 banks.
    with (
        tc.tile_pool(name="attn_wt_ps", bufs=2, space="PSUM") as wt_pool,
        tc.tile_pool(name="attn_ot_ps", bufs=1, space="PSUM") as ot_pool,
        tc.tile_pool(name="attn_sc", bufs=4) as sc_pool,
        tc.tile_pool(name="attn_rs", bufs=8) as rs_pool,
        tc.tile_pool(name="attn_o", bufs=3) as o_pool,
    ):
        for p in range(NPAIR):
            ot = ot_pool.tile([128, S], F32, tag="ot", name="ot")
            for kt in range(KT):
                ksl = slice(128 * kt, 128 * kt + 128)
                dq = kt // 4 if causal else 0
                r128 = kt % 4 if causal else 0
                win = 128 * r128 + 128
                heads = {}
                for hi, (plo, phi) in enumerate(((0, 64), (64, 128))):
                    score = sc_pool.tile([128, S], BF16, tag=f"sc{hi}",
                                         name=f"sc{hi}")
                    rsh = rs_pool.tile([128, 2], F32, tag=f"rsh{hi}",
                                       name=f"rsh{hi}")
                    nhalf = 0
                    for half in range(2):
                        # live columns start at the 128-block diagonal edge
                        lo = max(512 * dq + 128 * r128, 1024 * half)
                        hhi = 1024 * (half + 1)
                        if lo >= hhi:
                            continue
                        base = 1024 * half
                        wt = wt_pool.tile([128, 1024], F32, tag="wt",
                                          name="wt")
                        for qc2 in range(max(dq, 2 * half), 2 * half + 2):
                            w0 = 512 * qc2 - base
                            q0 = 512 * qc2
                            if causal and qc2 == dq:
                                # diagonal 128-block: additive mask (resets
                                # PSUM), QK accumulates on top; then plain
                                # QK for the fully-live suffix
                                nc.tensor.matmul(
                                    wt[:, w0 + 128 * r128:w0 + win],
                                    ident_sb[:], mask_tiles[kt][:],
                                    start=True, stop=False)
                                nc.tensor.matmul(
                                    wt[:, w0 + 128 * r128:w0 + win],
                                    kvt_sb[p][plo:phi, ksl],
                                    qt_sb[p][plo:phi,
                                             q0 + 128 * r128:q0 + win],
                                    start=False, stop=True,
                                    tile_position=(plo, 0))
                                if win < 512:
                                    nc.tensor.matmul(
                                        wt[:, w0 + win:w0 + 512],
                                        kvt_sb[p][plo:phi, ksl],
                                        qt_sb[p][plo:phi, q0 + win:q0 + 512],
                                        start=True, stop=True,
                                        tile_position=(plo, 0))
                            else:
                                nc.tensor.matmul(
                                    wt[:, w0:w0 + 512],
                                    kvt_sb[p][plo:phi, ksl],
                                    qt_sb[p][plo:phi, q0:q0 + 512],
                                    start=True, stop=True,
                                    tile_position=(plo, 0))
                        nc.scalar.activation(
                            score[:, lo:hhi], wt[:, lo - base:1024],
                            Exp, accum_out=rsh[:, nhalf:nhalf + 1])
                        nhalf += 1
                    if nhalf == 2:
                        rs = rs_pool.tile([128, 1], F32, tag=f"rs{hi}",
                                          name=f"rs{hi}")
                        nc.vector.tensor_add(rs[:], rsh[:, 0:1], rsh[:, 1:2])
                        rs_ap = rs[:]
                    else:
                        rs_ap = rsh[:, 0:1]
                    ri = rs_pool.tile([128, 1], F32, tag=f"ri{hi}",
                                      name=f"ri{hi}")
                    nc.vector.reciprocal(ri[:], rs_ap)
                    kvs = rs_pool.tile([128, DH], BF16, tag=f"kvs{hi}",
                                       name=f"kvs{hi}")
                    h_local = 2 * p + hi
                    nc.vector.tensor_scalar_mul(
                        kvs[:], kv_sb[kt][:, DH * h_local:DH * h_local + DH],
                        ri[:])
                    heads[hi] = (score, kvs, dq)
                for hi, (plo, phi) in enumerate(((0, 64), (64, 128))):
                    score, kvs, dq = heads[hi]
                    for qc2 in range(dq, QC):
                        q0 = 512 * qc2
                        # on the diagonal k-tile, skip the score columns
                        # left of the 128-block edge (zero / never written);
                        # they were started by earlier k-tiles
                        c0 = q0 + 128 * r128 if (causal and qc2 == dq) else q0
                        stop_kt = 4 * qc2 + 3 if causal else KT - 1
                        nc.tensor.matmul(ot[plo:phi, c0:q0 + 512], kvs[:],
                                         score[:, c0:q0 + 512],
                                         start=(kt == 0),
                                         stop=(kt == stop_kt),
                                         skip_group_check=causal,
                                         tile_position=(0, plo))
            o_sb = o_pool.tile([128, S], BF16, tag="o", name="o")
            nc.vector.tensor_copy(o_sb[:], ot[:])
            nc.sync.dma_start(ag_in[128 * p:128 * p + 128, :], o_sb[:])
            if on_pair is not None:
                on_pair(p)


def build(nc, stage="full", reps=1):
    xTb = nc.dram_tensor("xTb", [D, S], BF16, kind="ExternalInput")
    xT = nc.dram_tensor("xT", [D, S], F32, kind="ExternalInput")
    maskc = nc.dram_tensor("maskc", [S, 128], BF16, kind="ExternalInput")
    ident = nc.dram_tensor("ident", [128, 128], BF16, kind="ExternalInput")
    wq1 = nc.dram_tensor("wq1", [D, HLOC * DH], BF16, kind="ExternalInput")
    wv1 = nc.dram_tensor("wv1", [D, HLOC * DH], BF16, kind="ExternalInput")
    wq2 = nc.dram_tensor("wq2", [D, HLOC * DH], BF16, kind="ExternalInput")
    wv2 = nc.dram_tensor("wv2", [D, HLOC * DH], BF16, kind="ExternalInput")
    w1 = nc.dram_tensor("w1", [D, DFF], BF16, kind="ExternalInput")
    w2 = nc.dram_tensor("w2", [DFF, D], BF16, kind="ExternalInput")
    b1c = nc.dram_tensor("b1c", [DFF, 1], F32, kind="ExternalInput")
    b2c = nc.dram_tensor("b2c", [D, 1], F32, kind="ExternalInput")
    gb1 = nc.dram_tensor("gb1", [2, D], BF16, kind="ExternalInput")
    gb2 = nc.dram_tensor("gb2", [2, D], BF16, kind="ExternalInput")
    gbf = nc.dram_tensor("gbf", [2, D], BF16, kind="ExternalInput")
    cb = nc.dram_tensor("cb", [1, 1], I32, kind="ExternalInput")
    rb = nc.dram_tensor("rb", [1, 1], I32, kind="ExternalInput")

    ag1_in = nc.dram_tensor("ag1_in", [NPAIR * 128, S], BF16)
    ag1_outs = [nc.dram_tensor(f"ag1_out{p}", [NCORES * 128, S], BF16,
                               addr_space="Shared") for p in range(NPAIR)]
    x2s = nc.dram_tensor("x2s", [D, S], BF16)
    ag2_in = nc.dram_tensor("ag2_in", [NPAIR * 128, S], BF16)
    ag2_outs = [nc.dram_tensor(f"ag2_out{p}", [NCORES * 128, S], BF16,
                               addr_space="Shared") for p in range(NPAIR)]

    if stage in ("x2", "b1", "b1nm"):
        dbg = nc.dram_tensor("dbg", [D, S], F32, kind="ExternalOutput")
    elif stage == "x3":
        dbg = nc.dram_tensor("dbg", [D, SH], F32, kind="ExternalOutput")
    outT = None
    if stage in ("full", "sim", "fullnc"):
        outT = nc.dram_tensor("outT", [D, SH], F32, kind="ExternalOutput")

    rg = RG

    with tile.TileContext(nc) as tc:
        cv = _scalar_from_input(nc, cb, SH)
        rv = _scalar_from_input(nc, rb, 256 * (B - 1))
        for _rep in range(reps):
            _build_body(tc, nc, stage, cv, rv, locals())


def _all_gather_pair(nc, stage, rg, ag_in, ag_out_p, p):
    """AllGather one head-pair's slice within the 2-core batch group
    (emitted as soon as pair p's o^T is in DRAM, so earlier pairs'
    exchange overlaps later pairs' compute)."""
    in_ap = ag_in[128 * p:128 * p + 128, :]
    if stage.startswith("sim") or stage.startswith("fullnc"):
        nc.sync.dma_start(ag_out_p[0:128, :], in_ap)
        nc.sync.dma_start(ag_out_p[128:256, :], in_ap)
    else:
        nc.gpsimd.collective_compute(
            "AllGather", mybir.AluOpType.bypass, replica_groups=rg,
            ins=[in_ap.opt()], outs=[ag_out_p[:].opt()])


def _build_body(tc, nc, stage, cv, rv, env):
    (xTb, xT, maskc, wq1, wv1, wq2, wv2, w1, w2, b1c, b2c, gb1, gb2,
     gbf, x2s, ag1_in, ag1_outs, ag2_in, ag2_outs, rg, ident) = (
        env["xTb"], env["xT"], env["maskc"], env["wq1"], env["wv1"],
        env["wq2"], env["wv2"], env["w1"], env["w2"], env["b1c"],
        env["b2c"], env["gb1"], env["gb2"], env["gbf"], env["x2s"],
        env["ag1_in"], env["ag1_outs"], env["ag2_in"], env["ag2_outs"],
        env["rg"], env["ident"])
    dbg = env.get("dbg")
    outT = env.get("outT")
    with ExitStack() as top:
        const_pool = top.enter_context(tc.tile_pool(name="const", bufs=1))
        ones_b = const_pool.tile([128, 1], BF16, tag="ones_b", name="ones_b")
        nc.vector.memset(ones_b[:], 1.0)
        gb_sb = {}
        for nm, dram in (("gb1", gb1), ("gb2", gb2), ("gbf", gbf)):
            t = const_pool.tile([2, D], BF16, tag=nm, name=nm)
            nc.scalar.dma_start(t[:], dram[:])
            gb_sb[nm] = t
        ident_sb = const_pool.tile([128, 128], BF16, tag="ident", name="ident")
        nc.scalar.dma_start(ident_sb[:], ident[:])

        # ---------------- block 1 ----------------
        # pools that outlive the block-1 scope (stack-ordered before it)
        wpool2 = top.enter_context(tc.tile_pool(name="w2p", bufs=1))
        x2b_pool = top.enter_context(tc.tile_pool(name="x2b", bufs=1))
        with ExitStack() as blk1_outer:
            xb_pool = blk1_outer.enter_context(
                tc.tile_pool(name="xb", bufs=1))
            xb = [xb_pool.tile([128, S], BF16, tag=f"x{dt}", name=f"x{dt}")
                  for dt in range(DT)]
            with ExitStack() as blk1:
                # load order: weights (small, needed first by the dt-outer
                # projection), then x, then mask (needed ~35us later)
                wpool = blk1.enter_context(tc.tile_pool(name="w1p", bufs=1))
                wq_sb, wv_sb = [], []
                for dt in range(DT):
                    wq = wpool.tile([128, HLOC * DH], BF16, tag=f"wq{dt}",
                                    name=f"wq{dt}")
                    nc.sync.dma_start(wq[:], wq1[128 * dt:128 * dt + 128, :])
                    wq_sb.append(wq)
                    wv = wpool.tile([128, HLOC * DH], BF16, tag=f"wv{dt}",
                                    name=f"wv{dt}")
                    nc.sync.dma_start(wv[:], wv1[128 * dt:128 * dt + 128, :])
                    wv_sb.append(wv)
                    nc.sync.dma_start(xb[dt][:],
                                      xTb[128 * dt:128 * dt + 128, :])
                xf = [xb_pool.tile([128, S], F32, tag=f"xf{dt}",
                                   name=f"xf{dt}") for dt in range(DT)]
                for dt in range(DT):
                    nc.sync.dma_start(xf[dt][:],
                                      xT[128 * dt:128 * dt + 128, :])
                m_tiles = None
                if stage != "b1nm":
                    mask_pool = blk1.enter_context(
                        tc.tile_pool(name="mask", bufs=1))
                    m_tiles = []
                    for kt in range(KT):
                        m = mask_pool.tile([128, 128], BF16, tag=f"m{kt}",
                                           name=f"m{kt}")
                        nc.sync.dma_start(
                            m[:], maskc[128 * kt:128 * kt + 128, :])
                        m_tiles.append(m)
                _attention(tc, blk1, xb, wq_sb, wv_sb, m_tiles, ag1_in,
                           ident_sb, causal=(stage != "b1nm"),
                           on_pair=lambda p: _all_gather_pair(
                               nc, stage, rg, ag1_in, ag1_outs[p], p))

            if stage in ("b1", "b1nm"):
                with tc.tile_pool(name="b1dbg", bufs=2) as dp:
                    for dt in range(DT):
                        t = dp.tile([128, S], BF16, tag="d", name="d")
                        nc.sync.dma_start(
                            t[:], ag1_outs[dt % NPAIR][
                                bass.ds(rv + 128 * (dt // NPAIR), 128), :])
                        tf = dp.tile([128, S], F32, tag="df", name="df")
                        nc.vector.tensor_copy(tf[:], t[:])
                        nc.sync.dma_start(dbg[128 * dt:128 * dt + 128, :],
                                          tf[:])
                return

            # ---------------- LN1 -> x2 ----------------
            # prefetch block-2 weights during the gather window
            w2q_sb, w2v_sb = [], []
            for dt in range(DT):
                wq = wpool2.tile([128, HLOC * DH], BF16, tag=f"wq{dt}",
                                 name=f"wq{dt}")
                nc.scalar.dma_start(wq[:], wq2[128 * dt:128 * dt + 128, :])
                w2q_sb.append(wq)
            for dt in range(DT):
                wv = wpool2.tile([128, HLOC * DH], BF16, tag=f"wv{dt}",
                                 name=f"wv{dt}")
                nc.scalar.dma_start(wv[:], wv2[128 * dt:128 * dt + 128, :])
                w2v_sb.append(wv)

            x2b = [x2b_pool.tile([128, S], BF16, tag=f"x2b{dt}",
                                 name=f"x2b{dt}") for dt in range(DT)]
            with ExitStack() as lctx:
                rp = lctx.enter_context(tc.tile_pool(name="ln1_r", bufs=1))
                tp = lctx.enter_context(tc.tile_pool(name="ln1_t", bufs=2))
                r_tiles = [None] * DT
                for i, dt in enumerate(DT_ORDER):
                    t1 = tp.tile([128, S], BF16, tag="ag", name="ag")
                    eng = nc.sync if i % 2 == 0 else nc.scalar
                    eng.dma_start(
                        t1[:], ag1_outs[dt % NPAIR][
                            bass.ds(rv + 128 * (dt // NPAIR), 128), :])
                    r = rp.tile([128, S], BF16, tag=f"r{dt}", name=f"r{dt}")
                    nc.vector.tensor_add(r[:], t1[:], xf[dt][:])
                    r_tiles[dt] = r
                _layernorm(tc, lctx, r_tiles, gb_sb["gb1"], ones_b, S,
                           None, x2b)

        # spill x2 for the LN2 residual read-back (dynamic column half)
        for dt in range(DT):
            nc.sync.dma_start(x2s[128 * dt:128 * dt + 128, :], x2b[dt][:])

        if stage == "x2":
            with tc.tile_pool(name="x2dbg", bufs=2) as dp:
                for dt in range(DT):
                    tf = dp.tile([128, S], F32, tag="df", name="df")
                    nc.vector.tensor_copy(tf[:], x2b[dt][:])
                    nc.sync.dma_start(dbg[128 * dt:128 * dt + 128, :], tf[:])
            return

        # ---------------- block 2 ----------------
        # prefetch FFN w1 + biases on the Act queue (idle during proj)
        b1_sb, b2_sb, w1_sb = [], [], []
        if stage in ("full", "sim", "fullnc"):
            b_pool = top.enter_context(tc.tile_pool(name="ffn_b", bufs=1))
            w1_pool = top.enter_context(tc.tile_pool(name="ffn_w1", bufs=1))
            for ft in range(FT):
                bt = b_pool.tile([128, 1], F32, tag=f"b1_{ft}",
                                 name=f"b1_{ft}")
                nc.scalar.dma_start(bt[:], b1c[128 * ft:128 * ft + 128, :])
                b1_sb.append(bt)
            for dt in range(DT):
                bt = b_pool.tile([128, 1], F32, tag=f"b2_{dt}",
                                 name=f"b2_{dt}")
                nc.scalar.dma_start(bt[:], b2c[128 * dt:128 * dt + 128, :])
                b2_sb.append(bt)
            for dt in range(DT):
                wt = w1_pool.tile([128, DFF], BF16, tag=f"w1_{dt}",
                                  name=f"w1_{dt}")
                nc.scalar.dma_start(wt[:], w1[128 * dt:128 * dt + 128, :])
                w1_sb.append(wt)

        with ExitStack() as blk2:
            _attention(tc, blk2, x2b, w2q_sb, w2v_sb, None, ag2_in,
                       ident_sb, causal=False,
                       on_pair=lambda p: _all_gather_pair(
                           nc, stage, rg, ag2_in, ag2_outs[p], p),
                       dt_order=DT_ORDER)

        # ---------------- LN2 -> x3 (token half) ----------------
        x3_pool = top.enter_context(tc.tile_pool(name="x3", bufs=1))
        x3b = [x3_pool.tile([128, SH], BF16, tag=f"x3b{dt}", name=f"x3b{dt}")
               for dt in range(DT)]
        with ExitStack() as lctx:
            rp = lctx.enter_context(tc.tile_pool(name="ln2_r", bufs=1))
            tp = lctx.enter_context(tc.tile_pool(name="ln2_t", bufs=2))
            r_tiles = [None] * DT
            for i, dt in enumerate(DT_ORDER):
                t1 = tp.tile([128, SH], BF16, tag="ag", name="ag")
                t2 = tp.tile([128, SH], BF16, tag="xres", name="xres")
                nc.sync.dma_start(
                    t1[:], ag2_outs[dt % NPAIR][
                        bass.ds(rv + 128 * (dt // NPAIR), 128),
                        bass.ds(cv, SH)])
                nc.scalar.dma_start(
                    t2[:], x2s[128 * dt:128 * dt + 128, bass.ds(cv, SH)])
                r = rp.tile([128, SH], BF16, tag=f"r{dt}", name=f"r{dt}")
                nc.vector.tensor_add(r[:], t1[:], t2[:])
                r_tiles[dt] = r
            _layernorm(tc, lctx, r_tiles, gb_sb["gb2"], ones_b, SH,
                       None, x3b)

        if stage == "x3":
            with tc.tile_pool(name="x3dbg", bufs=2) as dp:
                for dt in range(DT):
                    tf = dp.tile([128, SH], F32, tag="df", name="df")
                    nc.vector.tensor_copy(tf[:], x3b[dt][:])
                    nc.sync.dma_start(dbg[128 * dt:128 * dt + 128, :], tf[:])
            return

        # ---------------- FFN ----------------
        r3_pool = top.enter_context(tc.tile_pool(name="r3", bufs=1))
        r3 = [r3_pool.tile([128, SH], F32, tag=f"r3{dt}", name=f"r3{dt}")
              for dt in range(DT)]
        with ExitStack() as ffn_stack:
            w2_pool = ffn_stack.enter_context(
                tc.tile_pool(name="ffn_w2", bufs=1))
            w2_sb = []
            for ft in range(FT):
                wt = w2_pool.tile([128, D], BF16, tag=f"w2_{ft}",
                                  name=f"w2_{ft}")
                nc.scalar.dma_start(wt[:], w2[128 * ft:128 * ft + 128, :])
                w2_sb.append(wt)
            h_pool = ffn_stack.enter_context(
                tc.tile_pool(name="ffn_h", bufs=3))
            with (
                tc.tile_pool(name="ffn_h_ps", bufs=2, space="PSUM") as hps,
                tc.tile_pool(name="ffn_y_ps", bufs=1, space="PSUM") as yps,
            ):
                for ch in range(SH // 512):
                    cs = slice(512 * ch, 512 * ch + 512)
                    y_ps = [yps.tile([128, 512], F32, tag=f"yp{dt}",
                                     name=f"yp{dt}") for dt in range(DT)]
                    for ft in range(FT):
                        ps = hps.tile([128, 512], F32, tag="hp", name="hp")
                        for i, dt in enumerate(DT_ORDER):
                            nc.tensor.matmul(
                                ps[:], w1_sb[dt][:, 128 * ft:128 * ft + 128],
                                x3b[dt][:, cs],
                                start=(i == 0), stop=(i == DT - 1))
                        h = h_pool.tile([128, 512], BF16, tag="h", name="h")
                        nc.vector.tensor_scalar(h[:], ps[:], b1_sb[ft][:],
                                                0.0, op0=Add, op1=Max)
                        for dt in range(DT):
                            nc.tensor.matmul(
                                y_ps[dt][:],
                                w2_sb[ft][:, 128 * dt:128 * dt + 128],
                                h[:],
                                start=(ft == 0), stop=(ft == FT - 1))
                    for dt in range(DT):
                        nc.vector.scalar_tensor_tensor(
                            r3[dt][:, cs], y_ps[dt][:], b2_sb[dt][:],
                            x3b[dt][:, cs], op0=Add, op1=Add)

        # ---------------- LN3 -> out ----------------
        with ExitStack() as lctx:
            ofin = [r3_pool.tile([128, SH], F32, tag=f"of{dt}",
                                 name=f"of{dt}") for dt in range(DT)]
            _layernorm(tc, lctx, r3, gb_sb["gbf"], ones_b, SH, ofin, None)
            for dt in range(DT):
                for ch in range(SH // 512):
                    cs = slice(512 * ch, 512 * ch + 512)
                    nc.sync.dma_start(outT[128 * dt:128 * dt + 128, cs],
                                      ofin[dt][:, cs])


_CACHE = {}


def _get_compiled(stage="full"):
    if stage not in _CACHE:
        reps = 1
        name = stage
        import re as _re
        m = _re.match(r"^(.*)_r(\d+)$", stage)
        if m:
            name, reps = m.group(1), int(m.group(2))
        ndev = 1 if name.startswith("sim") else NCORES
        nc = bacc.Bacc("TRN2", target_bir_lowering=False, debug=False,
                       num_devices=ndev)
        build(nc, name, reps=reps)
        nc.compile()
        _CACHE[stage] = nc
    return _CACHE[stage]


def make_in_maps(x, mask, Wq1, Wv1, g1, be1, Wq2, Wv2, g2, be2,
                 Wf1, bf1, Wf2, bf2, gf, bef):
    x = np.asarray(x, np.float32)
    mask = np.asarray(mask)
    maskT = np.where(np.asarray(mask[0, 0]).T, np.float32(-1e9),
                     np.float32(0.0))
    # per-k-tile diagonal 128-block of the additive mask
    maskc = np.empty((S, 128), np.float32)
    for kt in range(KT):
        c0 = 128 * kt
        maskc[128 * kt:128 * kt + 128] = maskT[128 * kt:128 * kt + 128,
                                               c0:c0 + 128]
    maskc = maskc.astype(NP_BF16)
    w1b = np.asarray(Wf1, np.float32).astype(NP_BF16)
    w2b = np.asarray(Wf2, np.float32).astype(NP_BF16)
    scale = np.float32(1.0 / SQRT_DK)
    in_maps = []
    for c in range(NCORES):
        b, hh = c // 2, c % 2
        cols = slice(HLOC * DH * hh, HLOC * DH * (hh + 1))
        xTf = np.ascontiguousarray(x[b].T)
        in_maps.append({
            "xTb": xTf.astype(NP_BF16),
            "xT": xTf,
            "ident": np.eye(128, dtype=np.float32).astype(NP_BF16),
            "maskc": maskc,
            # fold the 1/sqrt(dk) into the Q projection
            "wq1": (np.ascontiguousarray(
                np.asarray(Wq1, np.float32)[:, cols]) * scale).astype(NP_BF16),
            "wv1": np.ascontiguousarray(
                np.asarray(Wv1, np.float32)[:, cols]).astype(NP_BF16),
            "wq2": (np.ascontiguousarray(
                np.asarray(Wq2, np.float32)[:, cols]) * scale).astype(NP_BF16),
            "wv2": np.ascontiguousarray(
                np.asarray(Wv2, np.float32)[:, cols]).astype(NP_BF16),
            "w1": w1b,
            "w2": w2b,
            "b1c": np.asarray(bf1, np.float32).reshape(DFF, 1),
            "b2c": np.asarray(bf2, np.float32).reshape(D, 1),
            "gb1": np.stack([np.asarray(g1, np.float32),
                             np.asarray(be1, np.float32)]).astype(NP_BF16),
            "gb2": np.stack([np.asarray(g2, np.float32),
                             np.asarray(be2, np.float32)]).astype(NP_BF16),
            "gbf": np.stack([np.asarray(gf, np.float32),
                             np.asarray(bef, np.float32)]).astype(NP_BF16),
            "cb": np.array([[SH * hh]], np.int32),
            "rb": np.array([[256 * b]], np.int32),
        })
    return in_maps


def run_spmd(in_maps, stage="full"):
    nc = _get_compiled(stage)
    return bass_utils.run_bass_kernel_spmd(nc, in_maps,
                                           core_ids=list(range(NCORES)))


def kernel(**inputs):
    in_maps = make_in_maps(**inputs)
    res = run_spmd(in_maps, "full")
    out = np.empty((B, S, D), np.float32)
    for c in range(NCORES):
        b, hh = c // 2, c % 2
        out[b, SH * hh:SH * (hh + 1), :] = res.results[c]["outT"].T
    return out


class _Runner:
    """Reusable jitted dispatcher (mirrors bass2jax.run_bass_via_pjrt's
    multi-core path) so repeated executions skip re-tracing and host
    transfers — used for timing."""

    def __init__(self, stage="full"):
        import jax
        from jax.sharding import Mesh, PartitionSpec
        from jax.experimental.shard_map import shard_map
        from concourse import bass2jax as b2j

        b2j.install_neuronx_cc_hook()
        nc = _get_compiled(stage)
        pname = (nc.partition_id_tensor.name
                 if nc.partition_id_tensor else None)
        in_names, out_names, out_avals = [], [], []
        for alloc in nc.m.functions[0].allocations:
            if not isinstance(alloc, mybir.MemoryLocationSet):
                continue
            name = alloc.memorylocations[0].name
            if alloc.kind == "ExternalInput":
                if name != pname:
                    in_names.append(name)
            elif alloc.kind == "ExternalOutput":
                out_names.append(name)
                out_avals.append(jax.core.ShapedArray(
                    tuple(alloc.tensor_shape), mybir.dt.np(alloc.dtype)))
        self.in_names, self.out_names = list(in_names), list(out_names)
        self.out_avals = out_avals
        all_in = in_names + out_names
        if pname is not None:
            all_in = all_in + [pname]
        n_params, n_outs = len(in_names), len(out_names)

        def _body(*args):
            operands = list(args)
            if pname is not None:
                operands.append(b2j.partition_id_tensor())
            outs = b2j._bass_exec_p.bind(
                *operands, out_avals=tuple(out_avals), in_names=tuple(all_in),
                out_names=tuple(out_names), lowering_input_output_aliases=(),
                sim_require_finite=True, sim_require_nnan=True, nc=nc)
            return tuple(outs)

        devices = jax.devices()[:NCORES]
        mesh = Mesh(np.asarray(devices), ("core",))
        in_specs = (PartitionSpec("core"),) * (n_params + n_outs)
        out_specs = (PartitionSpec("core"),) * n_outs
        self.fn = jax.jit(
            shard_map(_body, mesh=mesh, in_specs=in_specs,
                      out_specs=out_specs, check_rep=False),
            donate_argnums=tuple(range(n_params, n_params + n_outs)),
            keep_unused=True)
        self._jax = jax

    def device_inputs(self, in_maps):
        import jax
        concat = [np.concatenate([np.asarray(in_maps[c][n])
                                  for c in range(NCORES)], axis=0)
                  for n in self.in_names]
        return [jax.device_put(a) for a in concat]

    def zero_outs(self):
        import jax.numpy as jnp
        return [jnp.zeros((NCORES * av.shape[0], *av.shape[1:]), av.dtype)
                for av in self.out_avals]

    def __call__(self, dev_in, zeros):
        return self.fn(*dev_in, *zeros)


class _RunnerNZ:
    """Timing runner: zero output buffers are created inside the shard_map
    body (device-local), so repeated calls move no host data at all."""

    def __init__(self, stage="full"):
        import jax
        import jax.numpy as jnp
        from jax.sharding import Mesh, PartitionSpec
        from jax.experimental.shard_map import shard_map
        from concourse import bass2jax as b2j

        b2j.install_neuronx_cc_hook()
        nc = _get_compiled(stage)
        pname = (nc.partition_id_tensor.name
                 if nc.partition_id_tensor else None)
        in_names, out_names, out_avals = [], [], []
        for alloc in nc.m.functions[0].allocations:
            if not isinstance(alloc, mybir.MemoryLocationSet):
                continue
            name = alloc.memorylocations[0].name
            if alloc.kind == "ExternalInput":
                if name != pname:
                    in_names.append(name)
            elif alloc.kind == "ExternalOutput":
                out_names.append(name)
                out_avals.append(jax.core.ShapedArray(
                    tuple(alloc.tensor_shape), mybir.dt.np(alloc.dtype)))
        self.in_names, self.out_names = in_names, out_names
        all_in = in_names + out_names
        if pname is not None:
            all_in = all_in + [pname]

        def _body(*args):
            operands = list(args)
            operands += [jnp.zeros(av.shape, av.dtype) for av in out_avals]
            if pname is not None:
                operands.append(b2j.partition_id_tensor())
            outs = b2j._bass_exec_p.bind(
                *operands, out_avals=tuple(out_avals), in_names=tuple(all_in),
                out_names=tuple(out_names), lowering_input_output_aliases=(),
                sim_require_finite=True, sim_require_nnan=True, nc=nc)
            return tuple(outs)

        devices = jax.devices()[:NCORES]
        mesh = Mesh(np.asarray(devices), ("core",))
        self.fn = jax.jit(
            shard_map(_body, mesh=mesh,
                      in_specs=(PartitionSpec("core"),) * len(in_names),
                      out_specs=(PartitionSpec("core"),) * len(out_names),
                      check_rep=False),
            keep_unused=True)

    def device_inputs(self, in_maps):
        import jax
        concat = [np.concatenate([np.asarray(in_maps[c][n])
                                  for c in range(NCORES)], axis=0)
                  for n in self.in_names]
        return [jax.device_put(a) for a in concat]

    def __call__(self, dev_in):
        return self.fn(*dev_in)
